# revision 5
# baseline (speedup 1.0000x reference)
"""Trainium2 Bass kernel for nn_PointTransformerBatchLayer.

Strategy (8 NeuronCores, no cross-core communication):
  - Shard the point axis N=1024 -> 128 points per core. Every core receives
    the full xyz / feats (needed for KNN and neighbor gathers) plus its own
    local slices.
  - Per core: KNN via one fp32 matmul (s = 2 p_i.p_j - |p_j|^2) + top-16
    through 2 rounds of DVE max/max_index/match_replace.
  - qkv projected with fp16 matmuls; k/v written to a DRAM row buffer
    [N, 12anchors x (k64|v64)] and gathered per anchor-pair with
    dma_gather(transpose=True), which lands [2a x 64ch] on partitions.
  - Anchors processed in pairs ("apairs") so all DVE/ACT work runs with 128
    busy partitions. pe' = pos_mlp2@relu(pos_mlp1@anchor_rot(rel)) + q is
    computed with block-diagonal weights + an identity-matmul broadcast of q.
  - softmax uses sum(attn)==1:  out = sum(e*(v_g+pe'))/sum(e) - q.
  - All intermediates fp16 (0.2% rel err), PSUM fp32, KNN fp32.
"""
import sys

if "/opt/trn_rl_repo" not in sys.path:
    sys.path.insert(0, "/opt/trn_rl_repo")

import numpy as np

import concourse.bass as bass
import concourse.bacc as bacc
import concourse.tile as tile
import concourse.mybir as mybir
from concourse.bass_utils import run_bass_kernel_spmd

F16 = mybir.dt.float16
F32 = mybir.dt.float32
I16 = mybir.dt.int16
U16 = mybir.dt.uint16
AF = mybir.ActivationFunctionType
ALU = mybir.AluOpType

DIM, N, KNN, NA = 64, 1024, 16, 12
NCORES, NLOC, NAP = 8, 128, 6
NK = NLOC * KNN            # 2048 gathered elements per core
ROW = NAP * 256            # kv row length in elements (fp16)
FC = 512                   # matmul free-dim chunk
NCHUNK = NK // FC          # 4 chunks per apair

_PROG = None               # cached (nc, input_names)


def _declare(nc):
    p = {}
    def inp(name, shape, dt):
        p[name] = nc.declare_dram_parameter(name, list(shape), dt, isOutput=False)
    inp("feats_stack", (NAP, 128, N), F16)
    inp("feats_loc", (NAP, 128, NLOC), F16)
    inp("xyz4_all", (4, N), F32)
    inp("xyz4_loc", (4, NLOC), F32)
    inp("xyz_loc3", (3, NLOC), F16)
    inp("xyz_rows", (N, 128), F16)
    inp("pos1_w", (NAP, 3, 128), F16)
    inp("w2blk", (128, 128), F16)
    inp("wq_blk", (128, 128), F16)
    inp("wkv_blk", (128, 256), F16)
    inp("w1t_rep", (128, 256), F16)
    inp("w2t", (128, 128), F16)
    inp("ident128", (128, 128), F16)
    inp("identT", (128, 128), F32)
    inp("repmat", (16, 128), F32)
    p["out"] = nc.declare_dram_parameter("out", [128, NAP * 128], F32, isOutput=True)
    return p


def _emit(tc, p):
    nc = tc.nc
    from contextlib import ExitStack

    with ExitStack() as ctx:
        const = ctx.enter_context(tc.tile_pool(name="const", bufs=1))
        big = ctx.enter_context(tc.tile_pool(name="big", bufs=1))
        work = ctx.enter_context(tc.tile_pool(name="work", bufs=3))
        work2 = ctx.enter_context(tc.tile_pool(name="work2", bufs=2))
        dram = ctx.enter_context(tc.tile_pool(name="dram", bufs=1, space="DRAM"))

        # ---- constant loads ----
        def load_const(name, shape, dt, ap=None):
            t = const.tile(list(shape), dt, tag=name)
            nc.sync.dma_start(t[:], ap if ap is not None else p[name][:])
            return t

        feats_sb = const.tile([128, NAP, N], F16, tag="feats")
        nc.sync.dma_start(feats_sb[:], p["feats_stack"].rearrange("a p n -> p a n"))
        featsl_sb = const.tile([128, NAP, NLOC], F16, tag="featsl")
        nc.sync.dma_start(featsl_sb[:], p["feats_loc"].rearrange("a p n -> p a n"))
        pos1_sb = const.tile([3, NAP, 128], F16, tag="pos1")
        nc.sync.dma_start(pos1_sb[:], p["pos1_w"].rearrange("a p n -> p a n"))

        xyz4a_sb = load_const("xyz4_all", (4, N), F32)
        xyz4l_sb = load_const("xyz4_loc", (4, NLOC), F32)
        xyzl3_sb = load_const("xyz_loc3", (3, NLOC), F16)
        w2blk_sb = load_const("w2blk", (128, 128), F16)
        wqblk_sb = load_const("wq_blk", (128, 128), F16)
        wkv_sb = load_const("wkv_blk", (128, 256), F16)
        w1t_sb = load_const("w1t_rep", (128, 256), F16)
        w2t_sb = load_const("w2t", (128, 128), F16)
        id16_sb = load_const("ident128", (128, 128), F16)
        idT_sb = load_const("identT", (128, 128), F32)
        rep_sb = load_const("repmat", (16, 128), F32)

        kv_rows = dram.tile([N, ROW], F16, tag="kv_rows")

        with tc.tile_pool(name="ps_pre", bufs=1, space="PSUM") as ps1, \
             tc.tile_pool(name="ps_qkv", bufs=2, space="PSUM") as psq:
            # ---- KNN ----
            s_ps = ps1.tile([128, N], F32, tag="s_ps")
            for i in range(2):
                nc.tensor.matmul(s_ps[:, i * 512:(i + 1) * 512], lhsT=xyz4l_sb[:],
                                 rhs=xyz4a_sb[:, i * 512:(i + 1) * 512],
                                 start=True, stop=True)
            s_sb = big.tile([128, N], F32, tag="s_sb")
            nc.scalar.activation(s_sb[:], s_ps[:], AF.Copy)

            m8a = big.tile([128, 8], F32, tag="m8a")
            m8b = big.tile([128, 8], F32, tag="m8b")
            idxu = big.tile([128, KNN], U16, tag="idxu")
            s2_sb = big.tile([128, N], F32, tag="s2_sb")
            nc.vector.max(m8a[:], s_sb[:])
            nc.vector.max_index(idxu[:, 0:8], m8a[:], s_sb[:])
            nc.vector.match_replace(s2_sb[:], m8a[:], s_sb[:], -1e30)
            nc.vector.max(m8b[:], s2_sb[:])
            nc.vector.max_index(idxu[:, 8:16], m8b[:], s2_sb[:])

            # idx -> wrapped [16,128] -> replicated [128,128] int16
            idxf = big.tile([128, KNN], F32, tag="idxf")
            nc.vector.tensor_copy(idxf[:], idxu[:])
            tp_ps = ps1.tile([16, 128], F32, tag="tp_ps")
            nc.tensor.transpose(tp_ps[:], idxf[:], idT_sb[:])
            idxT = big.tile([16, 128], F32, tag="idxT")
            nc.scalar.activation(idxT[:], tp_ps[:], AF.Copy)
            rep_ps = ps1.tile([128, 128], F32, tag="rep_ps")
            nc.tensor.matmul(rep_ps[:], lhsT=rep_sb[:], rhs=idxT[:], start=True, stop=True)
            idx_rep = big.tile([128, 128], I16, tag="idx_rep")
            nc.vector.tensor_copy(idx_rep[:], rep_ps[:])

            # ---- qkv ----
            q_sb = big.tile([128, NAP, NLOC], F16, tag="q_sb")
            for ap in range(NAP):
                q_ps = psq.tile([128, NLOC], F32, tag="q_ps")
                nc.tensor.matmul(q_ps[:], lhsT=wqblk_sb[:], rhs=featsl_sb[:, ap, :],
                                 start=True, stop=True)
                nc.scalar.activation(q_sb[:, ap, :], q_ps[:], AF.Copy)

                stage = work2.tile([128, 8, 256], F16, tag="stage")
                for cc in range(4):  # 2 n-chunks of 128 per psum tile
                    kv_ps = psq.tile([128, 512], F32, tag="kv_ps")
                    for j in range(2):
                        c = cc * 2 + j
                        nc.tensor.matmul(kv_ps[:, j * 256:(j + 1) * 256],
                                         lhsT=feats_sb[:, ap, c * 128:(c + 1) * 128],
                                         rhs=wkv_sb[:], start=True, stop=True)
                    nc.scalar.activation(stage[:, cc * 2:(cc + 1) * 2, :], kv_ps[:], AF.Copy)
                nc.sync.dma_start(
                    kv_rows.rearrange("(c p) (a e) -> p c a e", p=128, e=256)[:, :, ap, :],
                    stage[:])

        # ---- gathers ----
        # The SWDGE descriptor ring fits ~512 descriptors per instruction
        # (1024+ wedges the device), so every gather is chunked at 512 idxs.
        GC = 512  # idxs per gather call
        xyzg = big.tile([128, NK], F16, tag="xyzg")
        for cc in range(NK // GC):
            nc.gpsimd.dma_gather(
                out_ap=xyzg[:, cc * GC:(cc + 1) * GC].rearrange("p (o j) -> p o j", o=1),
                in_ap=p["xyz_rows"][:, :],
                idxs_ap=idx_rep[:, cc * (GC // 16):(cc + 1) * (GC // 16)],
                num_idxs=GC, num_idxs_reg=GC, elem_size=128,
                transpose=True, queue_num=0)

        # per-(apair, chunk) gather tiles [128, 2*GC]: [:, :GC]=k, [:, GC:]=v
        kvg = [[None] * NCHUNK for _ in range(NAP)]
        for ap in range(NAP):
            for cc in range(NCHUNK):
                t = work2.tile([128, 2 * GC], F16, tag=f"kvg{ap % 3}_{cc}")
                nc.gpsimd.dma_gather(
                    out_ap=t.rearrange("p (o j) -> p o j", o=2),
                    in_ap=kv_rows[:, ap * 256:(ap + 1) * 256],
                    idxs_ap=idx_rep[:, cc * (GC // 16):(cc + 1) * (GC // 16)],
                    num_idxs=GC, num_idxs_reg=GC, elem_size=256, elem_step=ROW,
                    transpose=True, queue_num=0)
                kvg[ap][cc] = t

        # ---- rel = xyz_loc (bcast over k) - gathered xyz ----
        rel_sb = big.tile([3, NK], F16, tag="rel")
        nc.vector.tensor_sub(
            rel_sb.rearrange("p (n k) -> p n k", k=KNN),
            xyzl3_sb.rearrange("p n -> p n ()").broadcast_to([3, NLOC, KNN]),
            xyzg[0:3, :].rearrange("p (n k) -> p n k", k=KNN))

        out_sb = big.tile([128, NAP * 128], F32, tag="out_sb")

        # ---- apair loop ----
        with tc.tile_pool(name="ps_main", bufs=2, space="PSUM") as ps2:
            for ap in range(NAP):
                e_sb = work2.tile([128, NK], F16, tag="e_sb")
                vgp_sb = work2.tile([128, NK], F16, tag="vgp_sb")

                for c in range(NCHUNK):
                    cs = slice(c * FC, (c + 1) * FC)
                    kg = kvg[ap][c][:, 0:FC]
                    vg = kvg[ap][c][:, FC:2 * FC]
                    # pos mlp1 + relu
                    pe1_ps = ps2.tile([128, FC], F32, tag="pe1")
                    nc.tensor.matmul(pe1_ps[:], lhsT=pos1_sb[:, ap, :], rhs=rel_sb[:, cs],
                                     start=True, stop=True)
                    relu1 = work.tile([128, FC], F16, tag="relu1")
                    nc.scalar.activation(relu1[:], pe1_ps[:], AF.Relu)
                    # pe' = W2blk @ relu1 + I @ q_bcast
                    pep_ps = ps2.tile([128, FC], F32, tag="pep")
                    nc.tensor.matmul(pep_ps[:], lhsT=w2blk_sb[:], rhs=relu1[:],
                                     start=True, stop=False)
                    nw = FC // KNN  # 32 points per chunk
                    qb = q_sb[:, ap, c * nw:(c + 1) * nw] \
                        .rearrange("p n -> p n ()").broadcast_to([128, nw, KNN])
                    nc.tensor.matmul(pep_ps.rearrange("p (n k) -> p n k", k=KNN),
                                     lhsT=id16_sb[:], rhs=qb, start=False, stop=True)
                    pep = work.tile([128, FC], F16, tag="pep_sb")
                    nc.scalar.activation(pep[:], pep_ps[:], AF.Copy)
                    # sim = pe' - k_g ; vgp = v_g + pe'
                    sim = work.tile([128, FC], F16, tag="sim")
                    nc.vector.tensor_sub(sim[:], pep[:], kg)
                    nc.vector.tensor_add(vgp_sb[:, cs], vg, pep[:])
                    # attn mlp1 (per anchor halves) + relu
                    hr = [[None, None], [None, None]]
                    for h in range(2):
                        for s in range(2):
                            h_ps = ps2.tile([128, FC], F32, tag="h_ps")
                            nc.tensor.matmul(
                                h_ps[:],
                                lhsT=w1t_sb[64 * h:64 * (h + 1), 128 * s:128 * (s + 1)],
                                rhs=sim[64 * h:64 * (h + 1), :],
                                start=True, stop=True)
                            t = work.tile([128, FC], F16, tag=f"hr{h}{s}")
                            if h == 0:
                                nc.scalar.activation(t[:], h_ps[:], AF.Relu)
                            else:
                                nc.vector.tensor_scalar_max(t[:], h_ps[:], 0.0)
                            hr[h][s] = t
                    # attn mlp2 (accumulate over the two 128-chunks)
                    s2_ps = ps2.tile([128, FC], F32, tag="s2_ps")
                    for h in range(2):
                        for s in range(2):
                            nc.tensor.matmul(s2_ps[64 * h:64 * (h + 1), :],
                                             lhsT=w2t_sb[:, 64 * s:64 * (s + 1)],
                                             rhs=hr[h][s][:],
                                             start=(s == 0), stop=(s == 1))
                    nc.scalar.activation(e_sb[:, cs], s2_ps[:], AF.Exp)

                # softmax-aggregate: out = sum(e*vgp)/sum(e) - q
                t_sb = work2.tile([128, NK], F16, tag="t_sb")
                nc.vector.tensor_mul(t_sb[:], e_sb[:], vgp_sb[:])
                S1 = work.tile([128, NLOC], F32, tag="S1")
                nc.vector.tensor_reduce(S1[:], t_sb.rearrange("p (n k) -> p n k", k=KNN),
                                        axis=mybir.AxisListType.X, op=ALU.add)
                S0 = work.tile([128, NLOC], F32, tag="S0")
                nc.vector.tensor_reduce(S0[:], e_sb.rearrange("p (n k) -> p n k", k=KNN),
                                        axis=mybir.AxisListType.X, op=ALU.add)
                r_sb = work.tile([128, NLOC], F32, tag="r_sb")
                nc.vector.reciprocal(r_sb[:], S0[:])
                o1 = work.tile([128, NLOC], F32, tag="o1")
                nc.vector.tensor_mul(o1[:], S1[:], r_sb[:])
                nc.vector.tensor_sub(out_sb[:, ap * 128:(ap + 1) * 128], o1[:], q_sb[:, ap, :])

        nc.sync.dma_start(p["out"][:, :], out_sb[:])


def build_program():
    global _PROG
    if _PROG is not None:
        return _PROG
    nc = bacc.Bacc("TRN2", target_bir_lowering=False, debug=False)
    p = _declare(nc)
    with tile.TileContext(nc) as tc:
        _emit(tc, p)
    nc.compile()
    _PROG = nc
    return nc


def host_prep(xyz, feats, anchors, to_qkv, pos_mlp1, pos_mlp2, attn_mlp1, attn_mlp2):
    f16, f32 = np.float16, np.float32
    xyz = np.asarray(xyz, f32)[0]        # [3, N]
    feats = np.asarray(feats, f32)[0]    # [DIM, N, NA]
    anchors = np.asarray(anchors, f32)
    to_qkv = np.asarray(to_qkv, f32)
    pos_mlp1 = np.asarray(pos_mlp1, f32)
    pos_mlp2 = np.asarray(pos_mlp2, f32)
    attn_mlp1 = np.asarray(attn_mlp1, f32)
    attn_mlp2 = np.asarray(attn_mlp2, f32)

    Wq, Wk, Wv = to_qkv[:DIM], to_qkv[DIM:2 * DIM], to_qkv[2 * DIM:]
    Wa = np.einsum("hj,aji->ahi", pos_mlp1, anchors)     # [NA, 64, 3]

    pos1_w = np.stack([np.concatenate([Wa[2 * ap].T, Wa[2 * ap + 1].T], axis=1)
                       for ap in range(NAP)]).astype(f16)
    w2blk = np.zeros((128, 128), f32)
    w2blk[:64, :64] = pos_mlp2.T
    w2blk[64:, 64:] = pos_mlp2.T
    wq_blk = np.zeros((128, 128), f32)
    wq_blk[:64, :64] = Wq.T
    wq_blk[64:, 64:] = Wq.T
    wkv_blk = np.zeros((128, 256), f32)
    wkv_blk[:64, 0:64] = Wk.T
    wkv_blk[64:, 64:128] = Wk.T
    wkv_blk[:64, 128:192] = Wv.T
    wkv_blk[64:, 192:256] = Wv.T
    w1t_rep = np.concatenate([attn_mlp1.T, attn_mlp1.T], axis=0)
    w2t = np.zeros((128, 128), f32)
    w2t[:, :64] = attn_mlp2.T[:128]
    w2t[:, 64:] = attn_mlp2.T[128:]
    repmat = np.zeros((16, 128), f32)
    repmat[np.arange(128) % 16, np.arange(128)] = 1.0

    feats_stack = np.zeros((NAP, 128, N), f32)
    for ap in range(NAP):
        feats_stack[ap, :64] = feats[:, :, 2 * ap]
        feats_stack[ap, 64:] = feats[:, :, 2 * ap + 1]
    feats_stack = feats_stack.astype(f16)

    xyz_rows = np.zeros((N, 128), f32)
    xyz_rows[:, :3] = xyz.T
    xyz_rows = xyz_rows.astype(f16)

    sq = np.sum(xyz * xyz, axis=0)
    xyz4_all = np.concatenate([xyz, sq[None]], axis=0).astype(f32)

    common = dict(
        feats_stack=feats_stack,
        xyz4_all=xyz4_all,
        xyz_rows=xyz_rows,
        pos1_w=pos1_w,
        w2blk=w2blk.astype(f16),
        wq_blk=wq_blk.astype(f16),
        wkv_blk=wkv_blk.astype(f16),
        w1t_rep=w1t_rep.astype(f16),
        w2t=w2t.astype(f16),
        ident128=np.eye(128, dtype=f16),
        identT=np.eye(128, dtype=f32),
        repmat=repmat,
    )
    per_core = []
    for core in range(NCORES):
        n0 = core * NLOC
        xyz4_loc = np.concatenate([2.0 * xyz[:, n0:n0 + NLOC],
                                   -np.ones((1, NLOC), f32)], axis=0).astype(f32)
        per_core.append(dict(
            common,
            feats_loc=np.ascontiguousarray(feats_stack[:, :, n0:n0 + NLOC]),
            xyz4_loc=xyz4_loc,
            xyz_loc3=xyz[:, n0:n0 + NLOC].astype(f16),
        ))
    return per_core


def assemble(outs):
    """outs: list of 8 arrays [128, 768] -> [1, 64, 1024, 12] fp32."""
    parts = []
    for o in outs:
        x = np.asarray(o, np.float32).reshape(2, 64, NAP, 128)
        parts.append(np.transpose(x, (1, 3, 2, 0)).reshape(64, 128, 12))
    return np.concatenate(parts, axis=1)[None].astype(np.float32)


def kernel(**inputs):
    nc = build_program()
    in_maps = host_prep(**inputs)
    res = run_bass_kernel_spmd(nc, in_maps, list(range(NCORES)))
    return assemble([res.results[i]["out"] for i in range(NCORES)])


# revision 15
# speedup vs baseline: 1.0417x; 1.0417x over previous
"""Trainium2 Bass kernel for nn_PointTransformerBatchLayer.

Strategy (8 NeuronCores, no cross-core communication):
  - Shard the point axis N=1024 -> 128 points per core. Every core receives
    the full xyz / feats (needed for KNN and neighbor gathers) plus its own
    local slices.
  - Per core: KNN via one fp32 matmul (s = 2 p_i.p_j - |p_j|^2) + top-16
    through 2 rounds of DVE max/max_index/match_replace.
  - qkv projected with bf16 matmuls; k/v for all anchors plus padded xyz are
    written to a DRAM row buffer [N, 12x(k64|v64) + xyz_pad] and gathered
    with dma_gather(transpose=True) in 4 x 512-idx calls (the SWDGE
    descriptor ring holds ~512 descriptors per instruction). One gather
    delivers every anchor's k/v and the neighbor xyz in the perfect
    (anchor-pair, channel)-on-partition layout.
  - Anchors processed in pairs ("apairs") so DVE/ACT run with 128 busy
    partitions. pe' = pos_mlp2@relu(pos_mlp1@anchor_rot(rel)) + q via
    block-diagonal weights + an identity-matmul broadcast of q.
  - softmax uses sum(attn)==1:  out = sum(e*(v_g+pe'))/sum(e) - q.
  - bf16 on the matmul path, fp16 on the exp/softmax path (0.5% rel err),
    PSUM fp32, KNN fp32.
"""
import sys

if "/opt/trn_rl_repo" not in sys.path:
    sys.path.insert(0, "/opt/trn_rl_repo")

import numpy as np

import concourse.bass as bass
import concourse.bacc as bacc
import concourse.tile as tile
import concourse.mybir as mybir
from concourse.bass_utils import run_bass_kernel_spmd

BF16 = mybir.dt.bfloat16
F16 = mybir.dt.float16
F32 = mybir.dt.float32
I16 = mybir.dt.int16
U16 = mybir.dt.uint16
AF = mybir.ActivationFunctionType
ALU = mybir.AluOpType

DIM, N, KNN, NA = 64, 1024, 16, 12
NCORES, NLOC, NAP = 8, 128, 6
NK = NLOC * KNN            # 2048 gathered elements per core
ROW = NAP * 256            # kv row: 12 anchors x (k64|v64) = 1536 elems
FC = 512                   # matmul free-dim chunk == gather chunk
NCHUNK = NK // FC          # 4 chunks
NQ = 4                     # SWDGE queues used for gathers

_PROG = None


def _declare(nc):
    p = {}
    def inp(name, shape, dt):
        p[name] = nc.declare_dram_parameter(name, list(shape), dt, isOutput=False)
    inp("feats_stack", (NAP, 128, N), BF16)
    inp("feats_loc", (NAP, 128, NLOC), BF16)
    inp("xyz4_all", (4, N), F32)
    inp("xyz4_loc", (4, NLOC), F32)
    inp("xyz_loc3", (3, NLOC), BF16)
    inp("xyz_rows", (N, 128), BF16)
    inp("pos1_w", (NAP, 3, 128), BF16)
    inp("w2blk", (128, 128), BF16)
    inp("wq_blk", (128, 128), BF16)
    inp("wkv_blk", (128, 256), BF16)
    inp("w1t_rep", (128, 256), BF16)
    inp("w2t", (128, 128), BF16)
    inp("ident128", (128, 128), BF16)
    inp("identT", (128, 128), F32)
    inp("repmat", (16, 128), F32)
    p["out"] = nc.declare_dram_parameter("out", [128, NAP * 128], F32, isOutput=True)
    return p


def _emit(tc, p):
    nc = tc.nc
    from contextlib import ExitStack

    with ExitStack() as ctx:
        const = ctx.enter_context(tc.tile_pool(name="const", bufs=1))
        big = ctx.enter_context(tc.tile_pool(name="big", bufs=1))
        work = ctx.enter_context(tc.tile_pool(name="work", bufs=3))
        work2 = ctx.enter_context(tc.tile_pool(name="work2", bufs=2))
        dram = ctx.enter_context(tc.tile_pool(name="dram", bufs=1, space="DRAM"))

        # ---- constant loads ----
        def load_const(name, shape, dt):
            t = const.tile(list(shape), dt, tag=name)
            nc.sync.dma_start(t[:], p[name][:])
            return t

        feats_sb = const.tile([128, NAP, N], BF16, tag="feats")
        nc.sync.dma_start(feats_sb[:], p["feats_stack"].rearrange("a p n -> p a n"))
        featsl_sb = const.tile([128, NAP, NLOC], BF16, tag="featsl")
        nc.sync.dma_start(featsl_sb[:], p["feats_loc"].rearrange("a p n -> p a n"))
        pos1_sb = const.tile([3, NAP, 128], BF16, tag="pos1")
        nc.sync.dma_start(pos1_sb[:], p["pos1_w"].rearrange("a p n -> p a n"))

        xyz4a_sb = load_const("xyz4_all", (4, N), F32)
        xyz4l_sb = load_const("xyz4_loc", (4, NLOC), F32)
        xyzl3_sb = load_const("xyz_loc3", (3, NLOC), BF16)
        w2blk_sb = load_const("w2blk", (128, 128), BF16)
        wqblk_sb = load_const("wq_blk", (128, 128), BF16)
        wkv_sb = load_const("wkv_blk", (128, 256), BF16)
        w1t_sb = load_const("w1t_rep", (128, 256), BF16)
        w2t_sb = load_const("w2t", (128, 128), BF16)
        id16_sb = load_const("ident128", (128, 128), BF16)
        idT_sb = load_const("identT", (128, 128), F32)
        rep_sb = load_const("repmat", (16, 128), F32)

        kv_rows = dram.tile([N, ROW], BF16, tag="kv_rows")

        with tc.tile_pool(name="ps_pre", bufs=1, space="PSUM") as ps1, \
             tc.tile_pool(name="ps_qkv", bufs=2, space="PSUM") as psq:
            # ---- KNN ----
            s_ps = ps1.tile([128, N], F32, tag="s_ps")
            for i in range(2):
                nc.tensor.matmul(s_ps[:, i * 512:(i + 1) * 512], lhsT=xyz4l_sb[:],
                                 rhs=xyz4a_sb[:, i * 512:(i + 1) * 512],
                                 start=True, stop=True)
            s_sb = big.tile([128, N], F32, tag="s_sb")
            nc.scalar.activation(s_sb[:], s_ps[:], AF.Copy)

            m8a = big.tile([128, 8], F32, tag="m8a")
            m8b = big.tile([128, 8], F32, tag="m8b")
            idxu = big.tile([128, KNN], U16, tag="idxu")
            s2_sb = big.tile([128, N], F32, tag="s2_sb")
            nc.vector.max(m8a[:], s_sb[:])
            nc.vector.max_index(idxu[:, 0:8], m8a[:], s_sb[:])
            nc.vector.match_replace(s2_sb[:], m8a[:], s_sb[:], -1e30)
            nc.vector.max(m8b[:], s2_sb[:])
            nc.vector.max_index(idxu[:, 8:16], m8b[:], s2_sb[:])

            # idx -> wrapped [16,128] -> replicated [128,128] int16
            idxf = big.tile([128, KNN], F32, tag="idxf")
            nc.vector.tensor_copy(idxf[:], idxu[:])
            tp_ps = ps1.tile([16, 128], F32, tag="tp_ps")
            nc.tensor.transpose(tp_ps[:], idxf[:], idT_sb[:])
            idxT = big.tile([16, 128], F32, tag="idxT")
            nc.scalar.activation(idxT[:], tp_ps[:], AF.Copy)
            rep_ps = ps1.tile([128, 128], F32, tag="rep_ps")
            nc.tensor.matmul(rep_ps[:], lhsT=rep_sb[:], rhs=idxT[:], start=True, stop=True)
            idx_rep = big.tile([128, 128], I16, tag="idx_rep")
            nc.vector.tensor_copy(idx_rep[:], rep_ps[:])

            # ---- qkv ----
            q_sb = big.tile([128, NAP, NLOC], BF16, tag="q_sb")
            for ap in range(NAP):
                q_ps = psq.tile([128, NLOC], F32, tag="q_ps")
                nc.tensor.matmul(q_ps[:], lhsT=wqblk_sb[:], rhs=featsl_sb[:, ap, :],
                                 start=True, stop=True)
                nc.scalar.activation(q_sb[:, ap, :], q_ps[:], AF.Copy)

                stage = work2.tile([128, 8, 256], BF16, tag="stage")
                for cc in range(4):  # 2 n-chunks of 128 per psum tile
                    kv_ps = psq.tile([128, 512], F32, tag="kv_ps")
                    for j in range(2):
                        c = cc * 2 + j
                        nc.tensor.matmul(kv_ps[:, j * 256:(j + 1) * 256],
                                         lhsT=feats_sb[:, ap, c * 128:(c + 1) * 128],
                                         rhs=wkv_sb[:], start=True, stop=True)
                    nc.scalar.activation(stage[:, cc * 2:(cc + 1) * 2, :], kv_ps[:], AF.Copy)
                nc.sync.dma_start(
                    kv_rows.rearrange("(c p) (a e) -> p c a e", p=128, e=256)[:, :, ap, :],
                    stage[:])

        # ---- gathers ----
        # SWDGE ring fits ~512 TX + ~1024 RX descriptors per instruction, so
        # every gather is a 512-idx chunk; calls round-robin the 4 SWDGE
        # queues, whose Q7 core-pairs generate descriptors in parallel.
        qrr = [0]
        def next_q():
            q = qrr[0] % NQ
            qrr[0] += 1
            return q

        xyzg = big.tile([128, NK], BF16, tag="xyzg")
        for cc in range(NCHUNK):
            nc.gpsimd.dma_gather(
                out_ap=xyzg[:, cc * FC:(cc + 1) * FC].rearrange("p (o j) -> p o j", o=1),
                in_ap=p["xyz_rows"][:, :],
                idxs_ap=idx_rep[:, cc * (FC // 16):(cc + 1) * (FC // 16)],
                num_idxs=FC, num_idxs_reg=FC, elem_size=128,
                transpose=True, queue_num=next_q())

        # per-(apair, chunk) gather tiles [128, 2*FC]: [:, :FC]=k, [:, FC:]=v
        kvg = [[None] * NCHUNK for _ in range(NAP)]
        for ap in range(NAP):
            for cc in range(NCHUNK):
                t = work2.tile([128, 2 * FC], BF16, tag=f"kvg{ap % 3}_{cc}")
                nc.gpsimd.dma_gather(
                    out_ap=t.rearrange("p (o j) -> p o j", o=2),
                    in_ap=kv_rows[:, ap * 256:(ap + 1) * 256],
                    idxs_ap=idx_rep[:, cc * (FC // 16):(cc + 1) * (FC // 16)],
                    num_idxs=FC, num_idxs_reg=FC, elem_size=256, elem_step=ROW,
                    transpose=True, queue_num=next_q())
                kvg[ap][cc] = t

        # ---- rel = xyz_loc (bcast over k) - gathered xyz ----
        nw = FC // KNN  # 32 points per chunk
        rel_sb = big.tile([3, NK], BF16, tag="rel")
        nc.vector.tensor_sub(
            rel_sb.rearrange("p (n k) -> p n k", k=KNN),
            xyzl3_sb.rearrange("p n -> p n ()").broadcast_to([3, NLOC, KNN]),
            xyzg[0:3, :].rearrange("p (n k) -> p n k", k=KNN))

        out_sb = big.tile([128, NAP * 128], F32, tag="out_sb")

        # ---- apair loop ----
        with tc.tile_pool(name="ps_m1", bufs=2, space="PSUM") as psm1, \
             tc.tile_pool(name="ps_m2", bufs=2, space="PSUM") as psm2:
            for ap in range(NAP):
                e_sb = work2.tile([128, NK], F16, tag="e_sb")
                vgp_sb = work2.tile([128, NK], F16, tag="vgp_sb")

                for c in range(NCHUNK):
                    cs = slice(c * FC, (c + 1) * FC)
                    kg = kvg[ap][c][:, 0:FC]
                    vg = kvg[ap][c][:, FC:2 * FC]
                    # pos mlp1 + relu
                    pe1_ps = psm1.tile([128, FC], F32, tag="pp")
                    nc.tensor.matmul(pe1_ps[:], lhsT=pos1_sb[:, ap, :], rhs=rel_sb[:, cs],
                                     start=True, stop=True)
                    relu1 = work.tile([128, FC], BF16, tag="relu1")
                    nc.scalar.activation(relu1[:], pe1_ps[:], AF.Relu)
                    # pe' = W2blk @ relu1 + I @ q_bcast
                    pep_ps = psm1.tile([128, FC], F32, tag="pp")
                    nc.tensor.matmul(pep_ps[:], lhsT=w2blk_sb[:], rhs=relu1[:],
                                     start=True, stop=False)
                    qb = q_sb[:, ap, c * nw:(c + 1) * nw] \
                        .rearrange("p n -> p n ()").broadcast_to([128, nw, KNN])
                    nc.tensor.matmul(pep_ps.rearrange("p (n k) -> p n k", k=KNN),
                                     lhsT=id16_sb[:], rhs=qb, start=False, stop=True)
                    pep = work.tile([128, FC], BF16, tag="pep_sb")
                    nc.scalar.activation(pep[:], pep_ps[:], AF.Copy)
                    # sim = pe' - k_g ; vgp = v_g + pe'
                    sim = work.tile([128, FC], BF16, tag="sim")
                    nc.vector.tensor_sub(sim[:], pep[:], kg)
                    nc.vector.tensor_add(vgp_sb[:, cs], vg, pep[:])
                    # attn mlp1 + mlp2, s-chunk at a time: the two anchor
                    # halves ride distinct PE row groups (mlp1) / col groups
                    # (mlp2), so adjacent issue runs them concurrently.
                    s2_ps = psm2.tile([128, FC], F32, tag="s2_ps")
                    for s in range(2):
                        hr = [None, None]
                        for h in range(2):
                            h_ps = psm1.tile([128, FC], F32, tag=f"h{h}")
                            nc.tensor.matmul(
                                h_ps[:],
                                lhsT=w1t_sb[64 * h:64 * (h + 1), 128 * s:128 * (s + 1)],
                                rhs=sim[64 * h:64 * (h + 1), :],
                                start=True, stop=True)
                            t = work.tile([128, FC], BF16, tag=f"hr{h}")
                            if h == 0:
                                nc.scalar.activation(t[:], h_ps[:], AF.Relu)
                            else:
                                nc.vector.tensor_scalar_max(t[:], h_ps[:], 0.0)
                            hr[h] = t
                        for h in range(2):
                            nc.tensor.matmul(s2_ps[64 * h:64 * (h + 1), :],
                                             lhsT=w2t_sb[:, 64 * s:64 * (s + 1)],
                                             rhs=hr[h][:],
                                             start=(s == 0), stop=(s == 1),
                                             skip_group_check=True)
                    nc.scalar.activation(e_sb[:, cs], s2_ps[:], AF.Exp)

                # softmax-aggregate: out = sum(e*vgp)/sum(e) - q
                t_sb = work2.tile([128, NK], F16, tag="t_sb")
                nc.vector.tensor_mul(t_sb[:], e_sb[:], vgp_sb[:])
                S1 = work.tile([128, NLOC], F32, tag="S1")
                nc.vector.tensor_reduce(S1[:], t_sb.rearrange("p (n k) -> p n k", k=KNN),
                                        axis=mybir.AxisListType.X, op=ALU.add)
                S0 = work.tile([128, NLOC], F32, tag="S0")
                nc.vector.tensor_reduce(S0[:], e_sb.rearrange("p (n k) -> p n k", k=KNN),
                                        axis=mybir.AxisListType.X, op=ALU.add)
                r_sb = work.tile([128, NLOC], F32, tag="r_sb")
                nc.vector.reciprocal(r_sb[:], S0[:])
                o1 = work.tile([128, NLOC], F32, tag="o1")
                nc.vector.tensor_mul(o1[:], S1[:], r_sb[:])
                nc.vector.tensor_sub(out_sb[:, ap * 128:(ap + 1) * 128], o1[:], q_sb[:, ap, :])

        nc.sync.dma_start(p["out"][:, :], out_sb[:])


def build_program():
    global _PROG
    if _PROG is not None:
        return _PROG
    nc = bacc.Bacc("TRN2", target_bir_lowering=False, debug=False,
                   num_swdge_queues=NQ)
    p = _declare(nc)
    with tile.TileContext(nc) as tc:
        _emit(tc, p)
    nc.compile()
    _PROG = nc
    return nc


def host_prep(xyz, feats, anchors, to_qkv, pos_mlp1, pos_mlp2, attn_mlp1, attn_mlp2):
    import ml_dtypes
    bf16, f32 = ml_dtypes.bfloat16, np.float32
    xyz = np.asarray(xyz, f32)[0]        # [3, N]
    feats = np.asarray(feats, f32)[0]    # [DIM, N, NA]
    anchors = np.asarray(anchors, f32)
    to_qkv = np.asarray(to_qkv, f32)
    pos_mlp1 = np.asarray(pos_mlp1, f32)
    pos_mlp2 = np.asarray(pos_mlp2, f32)
    attn_mlp1 = np.asarray(attn_mlp1, f32)
    attn_mlp2 = np.asarray(attn_mlp2, f32)

    Wq, Wk, Wv = to_qkv[:DIM], to_qkv[DIM:2 * DIM], to_qkv[2 * DIM:]
    Wa = np.einsum("hj,aji->ahi", pos_mlp1, anchors)     # [NA, 64, 3]

    pos1_w = np.stack([np.concatenate([Wa[2 * ap].T, Wa[2 * ap + 1].T], axis=1)
                       for ap in range(NAP)]).astype(bf16)
    w2blk = np.zeros((128, 128), f32)
    w2blk[:64, :64] = pos_mlp2.T
    w2blk[64:, 64:] = pos_mlp2.T
    wq_blk = np.zeros((128, 128), f32)
    wq_blk[:64, :64] = Wq.T
    wq_blk[64:, 64:] = Wq.T
    wkv_blk = np.zeros((128, 256), f32)
    wkv_blk[:64, 0:64] = Wk.T
    wkv_blk[64:, 64:128] = Wk.T
    wkv_blk[:64, 128:192] = Wv.T
    wkv_blk[64:, 192:256] = Wv.T
    w1t_rep = np.concatenate([attn_mlp1.T, attn_mlp1.T], axis=0)
    w2t = np.zeros((128, 128), f32)
    w2t[:, :64] = attn_mlp2.T[:128]
    w2t[:, 64:] = attn_mlp2.T[128:]
    repmat = np.zeros((16, 128), f32)
    repmat[np.arange(128) % 16, np.arange(128)] = 1.0

    feats_stack = np.zeros((NAP, 128, N), f32)
    for ap in range(NAP):
        feats_stack[ap, :64] = feats[:, :, 2 * ap]
        feats_stack[ap, 64:] = feats[:, :, 2 * ap + 1]
    feats_stack = feats_stack.astype(bf16)

    xyz_rows = np.zeros((N, 128), f32)
    xyz_rows[:, :3] = xyz.T
    xyz_rows = xyz_rows.astype(bf16)

    sq = np.sum(xyz * xyz, axis=0)
    xyz4_all = np.concatenate([xyz, sq[None]], axis=0).astype(f32)

    common = dict(
        feats_stack=feats_stack,
        xyz4_all=xyz4_all,
        xyz_rows=xyz_rows,
        pos1_w=pos1_w,
        w2blk=w2blk.astype(bf16),
        wq_blk=wq_blk.astype(bf16),
        wkv_blk=wkv_blk.astype(bf16),
        w1t_rep=w1t_rep.astype(bf16),
        w2t=w2t.astype(bf16),
        ident128=np.eye(128).astype(bf16),
        identT=np.eye(128, dtype=f32),
        repmat=repmat,
    )
    per_core = []
    for core in range(NCORES):
        n0 = core * NLOC
        xyz4_loc = np.concatenate([2.0 * xyz[:, n0:n0 + NLOC],
                                   -np.ones((1, NLOC), f32)], axis=0).astype(f32)
        per_core.append(dict(
            common,
            feats_loc=np.ascontiguousarray(feats_stack[:, :, n0:n0 + NLOC]),
            xyz4_loc=xyz4_loc,
            xyz_loc3=xyz[:, n0:n0 + NLOC].astype(bf16),
        ))
    return per_core


def assemble(outs):
    """outs: list of 8 arrays [128, 768] -> [1, 64, 1024, 12] fp32."""
    parts = []
    for o in outs:
        x = np.asarray(o, np.float32).reshape(2, 64, NAP, 128)
        parts.append(np.transpose(x, (1, 3, 2, 0)).reshape(64, 128, 12))
    return np.concatenate(parts, axis=1)[None].astype(np.float32)


def kernel(**inputs):
    nc = build_program()
    in_maps = host_prep(**inputs)
    res = run_bass_kernel_spmd(nc, in_maps, list(range(NCORES)))
    return assemble([res.results[i]["out"] for i in range(NCORES)])


# revision 16
# speedup vs baseline: 1.2643x; 1.2137x over previous
"""Trainium2 Bass kernel for nn_PointTransformerBatchLayer.

Strategy (8 NeuronCores, no cross-core communication):
  - Shard the point axis N=1024 -> 128 points per core. Every core receives
    the full xyz / feats (needed for KNN and neighbor gathers) plus its own
    local slices.
  - Per core: KNN via one fp32 matmul (s = 2 p_i.p_j - |p_j|^2) + top-16
    through 2 rounds of DVE max/max_index/match_replace.
  - qkv projected with bf16 matmuls; k/v for all anchors plus padded xyz are
    written to a DRAM row buffer [N, 12x(k64|v64) + xyz_pad] and gathered
    with dma_gather(transpose=True) in 4 x 512-idx calls (the SWDGE
    descriptor ring holds ~512 descriptors per instruction). One gather
    delivers every anchor's k/v and the neighbor xyz in the perfect
    (anchor-pair, channel)-on-partition layout.
  - Anchors processed in pairs ("apairs") so DVE/ACT run with 128 busy
    partitions. pe' = pos_mlp2@relu(pos_mlp1@anchor_rot(rel)) + q via
    block-diagonal weights + an identity-matmul broadcast of q.
  - softmax uses sum(attn)==1:  out = sum(e*(v_g+pe'))/sum(e) - q.
  - bf16 on the matmul path, fp16 on the exp/softmax path (0.5% rel err),
    PSUM fp32, KNN fp32.
"""
import sys

if "/opt/trn_rl_repo" not in sys.path:
    sys.path.insert(0, "/opt/trn_rl_repo")

import numpy as np

import concourse.bass as bass
import concourse.bacc as bacc
import concourse.tile as tile
import concourse.mybir as mybir
from concourse.bass_utils import run_bass_kernel_spmd

BF16 = mybir.dt.bfloat16
F16 = mybir.dt.float16
F32 = mybir.dt.float32
I16 = mybir.dt.int16
U16 = mybir.dt.uint16
AF = mybir.ActivationFunctionType
ALU = mybir.AluOpType

DIM, N, KNN, NA = 64, 1024, 16, 12
NCORES, NLOC, NAP = 8, 128, 6
NK = NLOC * KNN            # 2048 gathered elements per core
ROW = NAP * 256            # kv row: 12 anchors x (k64|v64) = 1536 elems
FC = 512                   # matmul free-dim chunk == gather chunk
NCHUNK = NK // FC          # 4 chunks
NQ = 4                     # SWDGE queues used for gathers

_PROG = None


def _declare(nc):
    p = {}
    def inp(name, shape, dt):
        p[name] = nc.declare_dram_parameter(name, list(shape), dt, isOutput=False)
    inp("feats_stack", (NAP, 128, N), BF16)
    inp("feats_loc", (NAP, 128, NLOC), BF16)
    inp("xyz4_all", (4, N), F32)
    inp("xyz4_loc", (4, NLOC), F32)
    inp("xyz_loc3", (3, NLOC), BF16)
    inp("xyz_rows", (N, 128), BF16)
    inp("pos1_w", (NAP, 3, 128), BF16)
    inp("w2blk", (128, 128), BF16)
    inp("wq_blk", (128, 128), BF16)
    inp("wkv_blk", (128, 256), BF16)
    inp("w1t_rep", (128, 256), BF16)
    inp("w2t", (128, 128), BF16)
    inp("ident128", (128, 128), BF16)
    inp("identT", (128, 128), F32)
    inp("repmat", (16, 128), F32)
    p["out"] = nc.declare_dram_parameter("out", [128, NAP * 128], F32, isOutput=True)
    return p


def _emit(tc, p):
    nc = tc.nc
    from contextlib import ExitStack

    with ExitStack() as ctx:
        const = ctx.enter_context(tc.tile_pool(name="const", bufs=1))
        big = ctx.enter_context(tc.tile_pool(name="big", bufs=1))
        work = ctx.enter_context(tc.tile_pool(name="work", bufs=3))
        work2 = ctx.enter_context(tc.tile_pool(name="work2", bufs=2))
        dram = ctx.enter_context(tc.tile_pool(name="dram", bufs=1, space="DRAM"))

        # ---- constant loads ----
        def load_const(name, shape, dt):
            t = const.tile(list(shape), dt, tag=name)
            nc.sync.dma_start(t[:], p[name][:])
            return t

        feats_sb = const.tile([128, NAP, N], BF16, tag="feats")
        nc.sync.dma_start(feats_sb[:], p["feats_stack"].rearrange("a p n -> p a n"))
        featsl_sb = const.tile([128, NAP, NLOC], BF16, tag="featsl")
        nc.sync.dma_start(featsl_sb[:], p["feats_loc"].rearrange("a p n -> p a n"))
        pos1_sb = const.tile([3, NAP, 128], BF16, tag="pos1")
        nc.sync.dma_start(pos1_sb[:], p["pos1_w"].rearrange("a p n -> p a n"))

        xyz4a_sb = load_const("xyz4_all", (4, N), F32)
        xyz4l_sb = load_const("xyz4_loc", (4, NLOC), F32)
        xyzl3_sb = load_const("xyz_loc3", (3, NLOC), BF16)
        w2blk_sb = load_const("w2blk", (128, 128), BF16)
        wqblk_sb = load_const("wq_blk", (128, 128), BF16)
        wkv_sb = load_const("wkv_blk", (128, 256), BF16)
        w1t_sb = load_const("w1t_rep", (128, 256), BF16)
        w2t_sb = load_const("w2t", (128, 128), BF16)
        id16_sb = load_const("ident128", (128, 128), BF16)
        idT_sb = load_const("identT", (128, 128), F32)
        rep_sb = load_const("repmat", (16, 128), F32)

        kv_rows = dram.tile([N, ROW], BF16, tag="kv_rows")

        with tc.tile_pool(name="ps_pre", bufs=1, space="PSUM") as ps1, \
             tc.tile_pool(name="ps_qkv", bufs=2, space="PSUM") as psq:
            # ---- KNN ----
            s_ps = ps1.tile([128, N], F32, tag="s_ps")
            for i in range(2):
                nc.tensor.matmul(s_ps[:, i * 512:(i + 1) * 512], lhsT=xyz4l_sb[:],
                                 rhs=xyz4a_sb[:, i * 512:(i + 1) * 512],
                                 start=True, stop=True)
            s_sb = big.tile([128, N], F32, tag="s_sb")
            nc.scalar.activation(s_sb[:], s_ps[:], AF.Copy)

            m8a = big.tile([128, 8], F32, tag="m8a")
            m8b = big.tile([128, 8], F32, tag="m8b")
            idxu = big.tile([128, KNN], U16, tag="idxu")
            s2_sb = big.tile([128, N], F32, tag="s2_sb")
            nc.vector.max(m8a[:], s_sb[:])
            nc.vector.max_index(idxu[:, 0:8], m8a[:], s_sb[:])
            nc.vector.match_replace(s2_sb[:], m8a[:], s_sb[:], -1e30)
            nc.vector.max(m8b[:], s2_sb[:])
            nc.vector.max_index(idxu[:, 8:16], m8b[:], s2_sb[:])

            # idx -> wrapped [16,128] -> replicated [128,128] int16
            idxf = big.tile([128, KNN], F32, tag="idxf")
            nc.vector.tensor_copy(idxf[:], idxu[:])
            tp_ps = ps1.tile([16, 128], F32, tag="tp_ps")
            nc.tensor.transpose(tp_ps[:], idxf[:], idT_sb[:])
            idxT = big.tile([16, 128], F32, tag="idxT")
            nc.scalar.activation(idxT[:], tp_ps[:], AF.Copy)
            rep_ps = ps1.tile([128, 128], F32, tag="rep_ps")
            nc.tensor.matmul(rep_ps[:], lhsT=rep_sb[:], rhs=idxT[:], start=True, stop=True)
            idx_rep = big.tile([128, 128], I16, tag="idx_rep")
            nc.vector.tensor_copy(idx_rep[:], rep_ps[:])

            # ---- qkv ----
            q_sb = big.tile([128, NAP, NLOC], BF16, tag="q_sb")
            for ap in range(NAP):
                q_ps = psq.tile([128, NLOC], F32, tag="q_ps")
                nc.tensor.matmul(q_ps[:], lhsT=wqblk_sb[:], rhs=featsl_sb[:, ap, :],
                                 start=True, stop=True)
                nc.scalar.activation(q_sb[:, ap, :], q_ps[:], AF.Copy)

                stage = work2.tile([128, 8, 256], BF16, tag="stage")
                for cc in range(4):  # 2 n-chunks of 128 per psum tile
                    kv_ps = psq.tile([128, 512], F32, tag="kv_ps")
                    for j in range(2):
                        c = cc * 2 + j
                        nc.tensor.matmul(kv_ps[:, j * 256:(j + 1) * 256],
                                         lhsT=feats_sb[:, ap, c * 128:(c + 1) * 128],
                                         rhs=wkv_sb[:], start=True, stop=True)
                    nc.scalar.activation(stage[:, cc * 2:(cc + 1) * 2, :], kv_ps[:], AF.Copy)
                nc.sync.dma_start(
                    kv_rows.rearrange("(c p) (a e) -> p c a e", p=128, e=256)[:, :, ap, :],
                    stage[:])

        # ---- gathers ----
        # SWDGE ring fits ~512 TX + ~1024 RX descriptors per instruction, so
        # every gather is a 512-idx chunk; calls round-robin the 4 SWDGE
        # queues, whose Q7 core-pairs generate descriptors in parallel.
        qrr = [0]
        def next_q():
            q = qrr[0] % NQ
            qrr[0] += 1
            return q

        xyzg = big.tile([128, NK], BF16, tag="xyzg")
        for cc in range(NCHUNK):
            nc.gpsimd.dma_gather(
                out_ap=xyzg[:, cc * FC:(cc + 1) * FC].rearrange("p (o j) -> p o j", o=1),
                in_ap=p["xyz_rows"][:, :],
                idxs_ap=idx_rep[:, cc * (FC // 16):(cc + 1) * (FC // 16)],
                num_idxs=FC, num_idxs_reg=FC, elem_size=128,
                transpose=True, queue_num=next_q())

        # per-(apair, chunk) gather tiles [128, 2*FC]: [:, :FC]=k, [:, FC:]=v
        kvg = [[None] * NCHUNK for _ in range(NAP)]
        for ap in range(NAP):
            for cc in range(NCHUNK):
                t = work2.tile([128, 2 * FC], BF16, tag=f"kvg{ap % 3}_{cc}")
                nc.gpsimd.dma_gather(
                    out_ap=t.rearrange("p (o j) -> p o j", o=2),
                    in_ap=kv_rows[:, ap * 256:(ap + 1) * 256],
                    idxs_ap=idx_rep[:, cc * (FC // 16):(cc + 1) * (FC // 16)],
                    num_idxs=FC, num_idxs_reg=FC, elem_size=256, elem_step=ROW,
                    transpose=True, queue_num=next_q())
                kvg[ap][cc] = t

        # ---- rel = xyz_loc (bcast over k) - gathered xyz ----
        nw = FC // KNN  # 32 points per chunk
        rel_sb = big.tile([3, NK], BF16, tag="rel")
        nc.vector.tensor_sub(
            rel_sb.rearrange("p (n k) -> p n k", k=KNN),
            xyzl3_sb.rearrange("p n -> p n ()").broadcast_to([3, NLOC, KNN]),
            xyzg[0:3, :].rearrange("p (n k) -> p n k", k=KNN))

        out_sb = big.tile([128, NAP * 128], F32, tag="out_sb")
        S1_all = big.tile([128, NAP * 128], F32, tag="S1_all")
        S0_all = big.tile([128, NAP * 128], F32, tag="S0_all")

        def tree16(dst, src):
            """dst[p, n] = sum over k of src[p, (n k)]; fp16 in, fp32 out."""
            a8 = work.tile([128, NLOC * 8], F16, tag="tr8")
            nc.vector.tensor_add(
                a8.rearrange("p (n k) -> p n k", k=8),
                src.rearrange("p (n k) -> p n k", k=KNN)[:, :, 0:8],
                src.rearrange("p (n k) -> p n k", k=KNN)[:, :, 8:16])
            a4 = work.tile([128, NLOC * 4], F16, tag="tr4")
            nc.vector.tensor_add(
                a4.rearrange("p (n k) -> p n k", k=4),
                a8.rearrange("p (n k) -> p n k", k=8)[:, :, 0:4],
                a8.rearrange("p (n k) -> p n k", k=8)[:, :, 4:8])
            a2 = work.tile([128, NLOC * 2], F16, tag="tr2")
            nc.vector.tensor_add(
                a2.rearrange("p (n k) -> p n k", k=2),
                a4.rearrange("p (n k) -> p n k", k=4)[:, :, 0:2],
                a4.rearrange("p (n k) -> p n k", k=4)[:, :, 2:4])
            nc.vector.tensor_add(
                dst,
                a2.rearrange("p (n k) -> p n k", k=2)[:, :, 0:1].rearrange("p n k -> p (n k)"),
                a2.rearrange("p (n k) -> p n k", k=2)[:, :, 1:2].rearrange("p n k -> p (n k)"))

        # ---- apair loop ----
        with tc.tile_pool(name="ps_m1", bufs=2, space="PSUM") as psm1, \
             tc.tile_pool(name="ps_m2", bufs=1, space="PSUM") as psmh:
            for ap in range(NAP):
                e_sb = work2.tile([128, NK], F16, tag="e_sb")
                vgp_sb = work2.tile([128, NK], F16, tag="vgp_sb")

                for c in range(NCHUNK):
                    cs = slice(c * FC, (c + 1) * FC)
                    kg = kvg[ap][c][:, 0:FC]
                    vg = kvg[ap][c][:, FC:2 * FC]
                    # pos mlp1 + relu
                    pe1_ps = psm1.tile([128, FC], F32, tag="pp")
                    nc.tensor.matmul(pe1_ps[:], lhsT=pos1_sb[:, ap, :], rhs=rel_sb[:, cs],
                                     start=True, stop=True)
                    relu1 = work.tile([128, FC], BF16, tag="relu1")
                    nc.scalar.activation(relu1[:], pe1_ps[:], AF.Relu)
                    # pe' = W2blk @ relu1 + I @ q_bcast   (stays in PSUM)
                    pep_ps = psm1.tile([128, FC], F32, tag="pp")
                    nc.tensor.matmul(pep_ps[:], lhsT=w2blk_sb[:], rhs=relu1[:],
                                     start=True, stop=False)
                    qb = q_sb[:, ap, c * nw:(c + 1) * nw] \
                        .rearrange("p n -> p n ()").broadcast_to([128, nw, KNN])
                    nc.tensor.matmul(pep_ps.rearrange("p (n k) -> p n k", k=KNN),
                                     lhsT=id16_sb[:], rhs=qb, start=False, stop=True)
                    # sim = pe' - k_g ; vgp = v_g + pe'  (both read pe' PSUM)
                    sim = work.tile([128, FC], BF16, tag="sim")
                    nc.vector.scalar_tensor_tensor(
                        sim[:], pep_ps[:], 0.0, kg, ALU.add, ALU.subtract)
                    nc.vector.scalar_tensor_tensor(
                        vgp_sb[:, cs], pep_ps[:], 0.0, vg, ALU.add, ALU.add)
                    # attn mlp1 + mlp2, s-chunk at a time; the two anchor
                    # halves ride distinct PE row groups (mlp1) / col groups
                    # (mlp2) and the h pair lands in one 2-bank PSUM tile so
                    # a single relu evacuates both.
                    s2_ps = psm1.tile([128, FC], F32, tag="s2_ps")
                    for s in range(2):
                        h_ps = psmh.tile([128, 2 * FC], F32, tag=f"hp{s}")
                        for h in range(2):
                            nc.tensor.matmul(
                                h_ps[:, h * FC:(h + 1) * FC],
                                lhsT=w1t_sb[64 * h:64 * (h + 1), 128 * s:128 * (s + 1)],
                                rhs=sim[64 * h:64 * (h + 1), :],
                                start=True, stop=True)
                        hr = work.tile([128, 2 * FC], BF16, tag=f"hr{s}")
                        if s == 0:
                            nc.scalar.activation(hr[:], h_ps[:], AF.Relu)
                        else:
                            nc.vector.tensor_scalar_max(hr[:], h_ps[:], 0.0)
                        for h in range(2):
                            nc.tensor.matmul(s2_ps[64 * h:64 * (h + 1), :],
                                             lhsT=w2t_sb[:, 64 * s:64 * (s + 1)],
                                             rhs=hr[:, h * FC:(h + 1) * FC],
                                             start=(s == 0), stop=(s == 1),
                                             skip_group_check=True)
                    nc.scalar.activation(e_sb[:, cs], s2_ps[:], AF.Exp)

                # softmax-aggregate: S1 = sum_k e*vgp, S0 = sum_k e
                t_sb = work2.tile([128, NK], F16, tag="t_sb")
                nc.vector.tensor_mul(t_sb[:], e_sb[:], vgp_sb[:])
                tree16(S1_all[:, ap * 128:(ap + 1) * 128], t_sb)
                tree16(S0_all[:, ap * 128:(ap + 1) * 128], e_sb)

        # out = S1/S0 - q   (one deferred pass over all apairs)
        r_all = big.tile([128, NAP * 128], F32, tag="r_all")
        nc.vector.reciprocal(r_all[:], S0_all[:])
        o_all = big.tile([128, NAP * 128], F32, tag="o_all")
        nc.vector.tensor_mul(o_all[:], S1_all[:], r_all[:])
        nc.vector.tensor_sub(out_sb[:], o_all[:],
                             q_sb.rearrange("p a n -> p (a n)"))

        nc.sync.dma_start(p["out"][:, :], out_sb[:])


def build_program():
    global _PROG
    if _PROG is not None:
        return _PROG
    nc = bacc.Bacc("TRN2", target_bir_lowering=False, debug=False,
                   num_swdge_queues=NQ)
    p = _declare(nc)
    with tile.TileContext(nc) as tc:
        _emit(tc, p)
    nc.compile()
    _PROG = nc
    return nc


def host_prep(xyz, feats, anchors, to_qkv, pos_mlp1, pos_mlp2, attn_mlp1, attn_mlp2):
    import ml_dtypes
    bf16, f32 = ml_dtypes.bfloat16, np.float32
    xyz = np.asarray(xyz, f32)[0]        # [3, N]
    feats = np.asarray(feats, f32)[0]    # [DIM, N, NA]
    anchors = np.asarray(anchors, f32)
    to_qkv = np.asarray(to_qkv, f32)
    pos_mlp1 = np.asarray(pos_mlp1, f32)
    pos_mlp2 = np.asarray(pos_mlp2, f32)
    attn_mlp1 = np.asarray(attn_mlp1, f32)
    attn_mlp2 = np.asarray(attn_mlp2, f32)

    Wq, Wk, Wv = to_qkv[:DIM], to_qkv[DIM:2 * DIM], to_qkv[2 * DIM:]
    Wa = np.einsum("hj,aji->ahi", pos_mlp1, anchors)     # [NA, 64, 3]

    pos1_w = np.stack([np.concatenate([Wa[2 * ap].T, Wa[2 * ap + 1].T], axis=1)
                       for ap in range(NAP)]).astype(bf16)
    w2blk = np.zeros((128, 128), f32)
    w2blk[:64, :64] = pos_mlp2.T
    w2blk[64:, 64:] = pos_mlp2.T
    wq_blk = np.zeros((128, 128), f32)
    wq_blk[:64, :64] = Wq.T
    wq_blk[64:, 64:] = Wq.T
    wkv_blk = np.zeros((128, 256), f32)
    wkv_blk[:64, 0:64] = Wk.T
    wkv_blk[64:, 64:128] = Wk.T
    wkv_blk[:64, 128:192] = Wv.T
    wkv_blk[64:, 192:256] = Wv.T
    w1t_rep = np.concatenate([attn_mlp1.T, attn_mlp1.T], axis=0)
    w2t = np.zeros((128, 128), f32)
    w2t[:, :64] = attn_mlp2.T[:128]
    w2t[:, 64:] = attn_mlp2.T[128:]
    repmat = np.zeros((16, 128), f32)
    repmat[np.arange(128) % 16, np.arange(128)] = 1.0

    feats_stack = np.zeros((NAP, 128, N), f32)
    for ap in range(NAP):
        feats_stack[ap, :64] = feats[:, :, 2 * ap]
        feats_stack[ap, 64:] = feats[:, :, 2 * ap + 1]
    feats_stack = feats_stack.astype(bf16)

    xyz_rows = np.zeros((N, 128), f32)
    xyz_rows[:, :3] = xyz.T
    xyz_rows = xyz_rows.astype(bf16)

    sq = np.sum(xyz * xyz, axis=0)
    xyz4_all = np.concatenate([xyz, sq[None]], axis=0).astype(f32)

    common = dict(
        feats_stack=feats_stack,
        xyz4_all=xyz4_all,
        xyz_rows=xyz_rows,
        pos1_w=pos1_w,
        w2blk=w2blk.astype(bf16),
        wq_blk=wq_blk.astype(bf16),
        wkv_blk=wkv_blk.astype(bf16),
        w1t_rep=w1t_rep.astype(bf16),
        w2t=w2t.astype(bf16),
        ident128=np.eye(128).astype(bf16),
        identT=np.eye(128, dtype=f32),
        repmat=repmat,
    )
    per_core = []
    for core in range(NCORES):
        n0 = core * NLOC
        xyz4_loc = np.concatenate([2.0 * xyz[:, n0:n0 + NLOC],
                                   -np.ones((1, NLOC), f32)], axis=0).astype(f32)
        per_core.append(dict(
            common,
            feats_loc=np.ascontiguousarray(feats_stack[:, :, n0:n0 + NLOC]),
            xyz4_loc=xyz4_loc,
            xyz_loc3=xyz[:, n0:n0 + NLOC].astype(bf16),
        ))
    return per_core


def assemble(outs):
    """outs: list of 8 arrays [128, 768] -> [1, 64, 1024, 12] fp32."""
    parts = []
    for o in outs:
        x = np.asarray(o, np.float32).reshape(2, 64, NAP, 128)
        parts.append(np.transpose(x, (1, 3, 2, 0)).reshape(64, 128, 12))
    return np.concatenate(parts, axis=1)[None].astype(np.float32)


def kernel(**inputs):
    nc = build_program()
    in_maps = host_prep(**inputs)
    res = run_bass_kernel_spmd(nc, in_maps, list(range(NCORES)))
    return assemble([res.results[i]["out"] for i in range(NCORES)])


# revision 19
# speedup vs baseline: 1.2705x; 1.0049x over previous
"""Trainium2 Bass kernel for nn_PointTransformerBatchLayer.

Strategy (8 NeuronCores, no cross-core communication):
  - Shard the point axis N=1024 -> 128 points per core. Every core receives
    the full xyz / feats (needed for KNN and neighbor gathers) plus its own
    local slices.
  - Per core: KNN via one fp32 matmul (s = 2 p_i.p_j - |p_j|^2) + top-16
    through 2 rounds of DVE max/max_index/match_replace.
  - qkv projected with bf16 matmuls; k/v for all anchors plus padded xyz are
    written to a DRAM row buffer [N, 12x(k64|v64) + xyz_pad] and gathered
    with dma_gather(transpose=True) in 4 x 512-idx calls (the SWDGE
    descriptor ring holds ~512 descriptors per instruction). One gather
    delivers every anchor's k/v and the neighbor xyz in the perfect
    (anchor-pair, channel)-on-partition layout.
  - Anchors processed in pairs ("apairs") so DVE/ACT run with 128 busy
    partitions. pe' = pos_mlp2@relu(pos_mlp1@anchor_rot(rel)) + q via
    block-diagonal weights + an identity-matmul broadcast of q.
  - softmax uses sum(attn)==1:  out = sum(e*(v_g+pe'))/sum(e) - q.
  - bf16 on the matmul path, fp16 on the exp/softmax path (0.5% rel err),
    PSUM fp32, KNN fp32.
"""
import sys

if "/opt/trn_rl_repo" not in sys.path:
    sys.path.insert(0, "/opt/trn_rl_repo")

import numpy as np

import concourse.bass as bass
import concourse.bacc as bacc
import concourse.tile as tile
import concourse.mybir as mybir
from concourse.bass_utils import run_bass_kernel_spmd

BF16 = mybir.dt.bfloat16
F16 = mybir.dt.float16
F32 = mybir.dt.float32
I16 = mybir.dt.int16
U16 = mybir.dt.uint16
AF = mybir.ActivationFunctionType
ALU = mybir.AluOpType

DIM, N, KNN, NA = 64, 1024, 16, 12
NCORES, NLOC, NAP = 8, 128, 6
NK = NLOC * KNN            # 2048 gathered elements per core
ROW = NAP * 256            # kv row: 12 anchors x (k64|v64) = 1536 elems
FC = 512                   # matmul free-dim chunk == gather chunk
NCHUNK = NK // FC          # 4 chunks
NQ = 4                     # SWDGE queues used for gathers

_PROG = None


def _declare(nc):
    p = {}
    def inp(name, shape, dt):
        p[name] = nc.declare_dram_parameter(name, list(shape), dt, isOutput=False)
    inp("feats_stack", (NAP, 128, N), BF16)
    inp("feats_loc", (NAP, 128, NLOC), BF16)
    inp("xyz4_all", (4, N), F32)
    inp("xyz4_loc", (4, NLOC), F32)
    inp("xyz_loc3", (3, NLOC), BF16)
    inp("xyz_rows", (N, 128), BF16)
    inp("pos1_w", (NAP, 3, 128), BF16)
    inp("w2blk", (128, 128), BF16)
    inp("wq_blk", (128, 128), BF16)
    inp("wkv_blk", (128, 256), BF16)
    inp("w1t_rep", (128, 256), BF16)
    inp("w2t", (128, 128), BF16)
    inp("ident128", (128, 128), BF16)
    inp("identT", (128, 128), F32)
    inp("repmat", (16, 128), F32)
    p["out"] = nc.declare_dram_parameter("out", [128, NAP * 128], F32, isOutput=True)
    return p


def _emit(tc, p):
    nc = tc.nc
    from contextlib import ExitStack

    with ExitStack() as ctx:
        const = ctx.enter_context(tc.tile_pool(name="const", bufs=1))
        big = ctx.enter_context(tc.tile_pool(name="big", bufs=1))
        work = ctx.enter_context(tc.tile_pool(name="work", bufs=3))
        work2 = ctx.enter_context(tc.tile_pool(name="work2", bufs=2))
        dram = ctx.enter_context(tc.tile_pool(name="dram", bufs=1, space="DRAM"))

        # ---- constant loads ----
        def load_const(name, shape, dt):
            t = const.tile(list(shape), dt, tag=name)
            nc.sync.dma_start(t[:], p[name][:])
            return t

        feats_sb = const.tile([128, NAP, N], BF16, tag="feats")
        for ap in range(NAP):
            nc.sync.dma_start(feats_sb[:, ap, :], p["feats_stack"][ap, :, :])
        featsl_sb = const.tile([128, NAP, NLOC], BF16, tag="featsl")
        nc.sync.dma_start(featsl_sb[:], p["feats_loc"].rearrange("a p n -> p a n"))
        pos1_sb = const.tile([3, NAP, 128], BF16, tag="pos1")
        nc.sync.dma_start(pos1_sb[:], p["pos1_w"].rearrange("a p n -> p a n"))

        xyz4a_sb = load_const("xyz4_all", (4, N), F32)
        xyz4l_sb = load_const("xyz4_loc", (4, NLOC), F32)
        xyzl3_sb = load_const("xyz_loc3", (3, NLOC), BF16)
        w2blk_sb = load_const("w2blk", (128, 128), BF16)
        wqblk_sb = load_const("wq_blk", (128, 128), BF16)
        wkv_sb = load_const("wkv_blk", (128, 256), BF16)
        w1t_sb = load_const("w1t_rep", (128, 256), BF16)
        w2t_sb = load_const("w2t", (128, 128), BF16)
        id16_sb = load_const("ident128", (128, 128), BF16)
        idT_sb = load_const("identT", (128, 128), F32)
        rep_sb = load_const("repmat", (16, 128), F32)

        kv_rows = dram.tile([N, ROW], BF16, tag="kv_rows")

        with tc.tile_pool(name="ps_pre", bufs=1, space="PSUM") as ps1, \
             tc.tile_pool(name="ps_qkv", bufs=2, space="PSUM") as psq:
            # ---- KNN ----
            s_ps = ps1.tile([128, N], F32, tag="s_ps")
            for i in range(2):
                nc.tensor.matmul(s_ps[:, i * 512:(i + 1) * 512], lhsT=xyz4l_sb[:],
                                 rhs=xyz4a_sb[:, i * 512:(i + 1) * 512],
                                 start=True, stop=True)
            s_sb = big.tile([128, N], F32, tag="s_sb")
            nc.scalar.activation(s_sb[:], s_ps[:], AF.Copy)

            m8a = big.tile([128, 8], F32, tag="m8a")
            m8b = big.tile([128, 8], F32, tag="m8b")
            idxu = big.tile([128, KNN], U16, tag="idxu")
            s2_sb = big.tile([128, N], F32, tag="s2_sb")
            nc.vector.max(m8a[:], s_sb[:])
            nc.vector.max_index(idxu[:, 0:8], m8a[:], s_sb[:])
            nc.vector.match_replace(s2_sb[:], m8a[:], s_sb[:], -1e30)
            nc.vector.max(m8b[:], s2_sb[:])
            nc.vector.max_index(idxu[:, 8:16], m8b[:], s2_sb[:])

            # idx -> wrapped [16,128] -> replicated [128,128] int16
            idxf = big.tile([128, KNN], F32, tag="idxf")
            nc.vector.tensor_copy(idxf[:], idxu[:])
            tp_ps = ps1.tile([16, 128], F32, tag="tp_ps")
            nc.tensor.transpose(tp_ps[:], idxf[:], idT_sb[:])
            idxT = big.tile([16, 128], F32, tag="idxT")
            nc.scalar.activation(idxT[:], tp_ps[:], AF.Copy)
            rep_ps = ps1.tile([128, 128], F32, tag="rep_ps")
            nc.tensor.matmul(rep_ps[:], lhsT=rep_sb[:], rhs=idxT[:], start=True, stop=True)
            idx_rep = big.tile([128, 128], I16, tag="idx_rep")
            nc.vector.tensor_copy(idx_rep[:], rep_ps[:])

            # ---- qkv ----
            q_sb = big.tile([128, NAP, NLOC], BF16, tag="q_sb")
            for ap in range(NAP):
                q_ps = psq.tile([128, NLOC], F32, tag="q_ps")
                nc.tensor.matmul(q_ps[:], lhsT=wqblk_sb[:], rhs=featsl_sb[:, ap, :],
                                 start=True, stop=True)
                nc.scalar.activation(q_sb[:, ap, :], q_ps[:], AF.Copy)

                stage = work2.tile([128, 8, 256], BF16, tag="stage")
                for cc in range(4):  # 2 n-chunks of 128 per psum tile
                    kv_ps = psq.tile([128, 512], F32, tag="kv_ps")
                    for j in range(2):
                        c = cc * 2 + j
                        nc.tensor.matmul(kv_ps[:, j * 256:(j + 1) * 256],
                                         lhsT=feats_sb[:, ap, c * 128:(c + 1) * 128],
                                         rhs=wkv_sb[:], start=True, stop=True)
                    nc.scalar.activation(stage[:, cc * 2:(cc + 1) * 2, :], kv_ps[:], AF.Copy)
                nc.sync.dma_start(
                    kv_rows.rearrange("(c p) (a e) -> p c a e", p=128, e=256)[:, :, ap, :],
                    stage[:])

        # ---- gathers ----
        # SWDGE ring fits ~512 TX + ~1024 RX descriptors per instruction, so
        # every gather is a 512-idx chunk; calls round-robin the 4 SWDGE
        # queues, whose Q7 core-pairs generate descriptors in parallel.
        qrr = [0]
        def next_q():
            q = qrr[0] % NQ
            qrr[0] += 1
            return q

        xyzg = big.tile([128, NK], BF16, tag="xyzg")
        for cc in range(NCHUNK):
            nc.gpsimd.dma_gather(
                out_ap=xyzg[:, cc * FC:(cc + 1) * FC].rearrange("p (o j) -> p o j", o=1),
                in_ap=p["xyz_rows"][:, :],
                idxs_ap=idx_rep[:, cc * (FC // 16):(cc + 1) * (FC // 16)],
                num_idxs=FC, num_idxs_reg=FC, elem_size=128,
                transpose=True, queue_num=next_q())

        # per-(apair, chunk) gather tiles [128, 2*FC]: [:, :FC]=k, [:, FC:]=v
        kvg = [[None] * NCHUNK for _ in range(NAP)]
        for ap in range(NAP):
            for cc in range(NCHUNK):
                t = work2.tile([128, 2 * FC], BF16, tag=f"kvg{ap % 3}_{cc}")
                nc.gpsimd.dma_gather(
                    out_ap=t.rearrange("p (o j) -> p o j", o=2),
                    in_ap=kv_rows[:, ap * 256:(ap + 1) * 256],
                    idxs_ap=idx_rep[:, cc * (FC // 16):(cc + 1) * (FC // 16)],
                    num_idxs=FC, num_idxs_reg=FC, elem_size=256, elem_step=ROW,
                    transpose=True, queue_num=next_q())
                kvg[ap][cc] = t

        # ---- rel = xyz_loc (bcast over k) - gathered xyz ----
        nw = FC // KNN  # 32 points per chunk
        rel_sb = big.tile([3, NK], BF16, tag="rel")
        nc.vector.tensor_sub(
            rel_sb.rearrange("p (n k) -> p n k", k=KNN),
            xyzl3_sb.rearrange("p n -> p n ()").broadcast_to([3, NLOC, KNN]),
            xyzg[0:3, :].rearrange("p (n k) -> p n k", k=KNN))

        out_sb = big.tile([128, NAP * 128], F32, tag="out_sb")
        S1_all = big.tile([128, NAP * 128], F32, tag="S1_all")
        S0_all = big.tile([128, NAP * 128], F32, tag="S0_all")

        def tree16(dst, src):
            """dst[p, n] = sum over k of src[p, (n k)]; fp16 in, fp32 out."""
            a8 = work.tile([128, NLOC * 8], F16, tag="tr8")
            nc.vector.tensor_add(
                a8.rearrange("p (n k) -> p n k", k=8),
                src.rearrange("p (n k) -> p n k", k=KNN)[:, :, 0:8],
                src.rearrange("p (n k) -> p n k", k=KNN)[:, :, 8:16])
            a4 = work.tile([128, NLOC * 4], F16, tag="tr4")
            nc.vector.tensor_add(
                a4.rearrange("p (n k) -> p n k", k=4),
                a8.rearrange("p (n k) -> p n k", k=8)[:, :, 0:4],
                a8.rearrange("p (n k) -> p n k", k=8)[:, :, 4:8])
            a2 = work.tile([128, NLOC * 2], F16, tag="tr2")
            nc.vector.tensor_add(
                a2.rearrange("p (n k) -> p n k", k=2),
                a4.rearrange("p (n k) -> p n k", k=4)[:, :, 0:2],
                a4.rearrange("p (n k) -> p n k", k=4)[:, :, 2:4])
            nc.vector.tensor_add(
                dst,
                a2.rearrange("p (n k) -> p n k", k=2)[:, :, 0:1].rearrange("p n k -> p (n k)"),
                a2.rearrange("p (n k) -> p n k", k=2)[:, :, 1:2].rearrange("p n k -> p (n k)"))

        # ---- apair loop ----
        with tc.tile_pool(name="ps_m1", bufs=2, space="PSUM") as psm1, \
             tc.tile_pool(name="ps_m2", bufs=1, space="PSUM") as psmh:
            for ap in range(NAP):
                e_sb = work2.tile([128, NK], F16, tag="e_sb")
                vgp_sb = work2.tile([128, NK], F16, tag="vgp_sb")

                for c in range(NCHUNK):
                    cs = slice(c * FC, (c + 1) * FC)
                    kg = kvg[ap][c][:, 0:FC]
                    vg = kvg[ap][c][:, FC:2 * FC]
                    # pos mlp1 + relu
                    pe1_ps = psm1.tile([128, FC], F32, tag="pp")
                    nc.tensor.matmul(pe1_ps[:], lhsT=pos1_sb[:, ap, :], rhs=rel_sb[:, cs],
                                     start=True, stop=True)
                    relu1 = work.tile([128, FC], BF16, tag="relu1")
                    nc.scalar.activation(relu1[:], pe1_ps[:], AF.Relu)
                    # pe' = W2blk @ relu1 + I @ q_bcast   (stays in PSUM)
                    pep_ps = psm1.tile([128, FC], F32, tag="pp")
                    nc.tensor.matmul(pep_ps[:], lhsT=w2blk_sb[:], rhs=relu1[:],
                                     start=True, stop=False)
                    qb = q_sb[:, ap, c * nw:(c + 1) * nw] \
                        .rearrange("p n -> p n ()").broadcast_to([128, nw, KNN])
                    nc.tensor.matmul(pep_ps.rearrange("p (n k) -> p n k", k=KNN),
                                     lhsT=id16_sb[:], rhs=qb, start=False, stop=True)
                    # sim = pe' - k_g ; vgp = v_g + pe'  (both read pe' PSUM)
                    sim = work.tile([128, FC], BF16, tag="sim")
                    nc.vector.scalar_tensor_tensor(
                        sim[:], pep_ps[:], 0.0, kg, ALU.add, ALU.subtract)
                    nc.vector.scalar_tensor_tensor(
                        vgp_sb[:, cs], pep_ps[:], 0.0, vg, ALU.add, ALU.add)
                    # attn mlp1 + mlp2, s-chunk at a time; the two anchor
                    # halves ride distinct PE row groups (mlp1) / col groups
                    # (mlp2) and the h pair lands in one 2-bank PSUM tile so
                    # a single relu evacuates both.
                    s2_ps = psm1.tile([128, FC], F32, tag="s2_ps")
                    for s in range(2):
                        h_ps = psmh.tile([128, 2 * FC], F32, tag=f"hp{s}")
                        for h in range(2):
                            nc.tensor.matmul(
                                h_ps[:, h * FC:(h + 1) * FC],
                                lhsT=w1t_sb[64 * h:64 * (h + 1), 128 * s:128 * (s + 1)],
                                rhs=sim[64 * h:64 * (h + 1), :],
                                start=True, stop=True)
                        hr = work.tile([128, 2 * FC], BF16, tag=f"hr{s}")
                        if (c + s) % 2 == 0:
                            nc.scalar.activation(hr[:], h_ps[:], AF.Relu)
                        else:
                            nc.vector.tensor_scalar_max(hr[:], h_ps[:], 0.0)
                        for h in range(2):
                            nc.tensor.matmul(s2_ps[64 * h:64 * (h + 1), :],
                                             lhsT=w2t_sb[:, 64 * s:64 * (s + 1)],
                                             rhs=hr[:, h * FC:(h + 1) * FC],
                                             start=(s == 0), stop=(s == 1),
                                             skip_group_check=True)
                    nc.scalar.activation(e_sb[:, cs], s2_ps[:], AF.Exp)

                # softmax-aggregate: S1 = sum_k e*vgp, S0 = sum_k e
                aps = slice(ap * 128, (ap + 1) * 128)
                t_sb = work2.tile([128, NK], F16, tag="t_sb")
                nc.vector.tensor_mul(t_sb[:], e_sb[:], vgp_sb[:])
                tree16(S1_all[:, aps], t_sb)
                tree16(S0_all[:, aps], e_sb)
                # out = S1/S0 - q, written out per apair to hide the tail
                nc.vector.reciprocal(S0_all[:, aps], S0_all[:, aps])
                nc.vector.tensor_mul(S1_all[:, aps], S1_all[:, aps], S0_all[:, aps])
                nc.vector.tensor_sub(out_sb[:, aps], S1_all[:, aps], q_sb[:, ap, :])
                nc.sync.dma_start(p["out"][:, ap * 128:(ap + 1) * 128], out_sb[:, aps])


def build_program():
    global _PROG
    if _PROG is not None:
        return _PROG
    nc = bacc.Bacc("TRN2", target_bir_lowering=False, debug=False,
                   num_swdge_queues=NQ)
    p = _declare(nc)
    with tile.TileContext(nc) as tc:
        _emit(tc, p)
    nc.compile()
    _PROG = nc
    return nc


def host_prep(xyz, feats, anchors, to_qkv, pos_mlp1, pos_mlp2, attn_mlp1, attn_mlp2):
    import ml_dtypes
    bf16, f32 = ml_dtypes.bfloat16, np.float32
    xyz = np.asarray(xyz, f32)[0]        # [3, N]
    feats = np.asarray(feats, f32)[0]    # [DIM, N, NA]
    anchors = np.asarray(anchors, f32)
    to_qkv = np.asarray(to_qkv, f32)
    pos_mlp1 = np.asarray(pos_mlp1, f32)
    pos_mlp2 = np.asarray(pos_mlp2, f32)
    attn_mlp1 = np.asarray(attn_mlp1, f32)
    attn_mlp2 = np.asarray(attn_mlp2, f32)

    Wq, Wk, Wv = to_qkv[:DIM], to_qkv[DIM:2 * DIM], to_qkv[2 * DIM:]
    Wa = np.einsum("hj,aji->ahi", pos_mlp1, anchors)     # [NA, 64, 3]

    pos1_w = np.stack([np.concatenate([Wa[2 * ap].T, Wa[2 * ap + 1].T], axis=1)
                       for ap in range(NAP)]).astype(bf16)
    w2blk = np.zeros((128, 128), f32)
    w2blk[:64, :64] = pos_mlp2.T
    w2blk[64:, 64:] = pos_mlp2.T
    wq_blk = np.zeros((128, 128), f32)
    wq_blk[:64, :64] = Wq.T
    wq_blk[64:, 64:] = Wq.T
    wkv_blk = np.zeros((128, 256), f32)
    wkv_blk[:64, 0:64] = Wk.T
    wkv_blk[64:, 64:128] = Wk.T
    wkv_blk[:64, 128:192] = Wv.T
    wkv_blk[64:, 192:256] = Wv.T
    w1t_rep = np.concatenate([attn_mlp1.T, attn_mlp1.T], axis=0)
    w2t = np.zeros((128, 128), f32)
    w2t[:, :64] = attn_mlp2.T[:128]
    w2t[:, 64:] = attn_mlp2.T[128:]
    repmat = np.zeros((16, 128), f32)
    repmat[np.arange(128) % 16, np.arange(128)] = 1.0

    feats_stack = np.zeros((NAP, 128, N), f32)
    for ap in range(NAP):
        feats_stack[ap, :64] = feats[:, :, 2 * ap]
        feats_stack[ap, 64:] = feats[:, :, 2 * ap + 1]
    feats_stack = feats_stack.astype(bf16)

    xyz_rows = np.zeros((N, 128), f32)
    xyz_rows[:, :3] = xyz.T
    xyz_rows = xyz_rows.astype(bf16)

    sq = np.sum(xyz * xyz, axis=0)
    xyz4_all = np.concatenate([xyz, sq[None]], axis=0).astype(f32)

    common = dict(
        feats_stack=feats_stack,
        xyz4_all=xyz4_all,
        xyz_rows=xyz_rows,
        pos1_w=pos1_w,
        w2blk=w2blk.astype(bf16),
        wq_blk=wq_blk.astype(bf16),
        wkv_blk=wkv_blk.astype(bf16),
        w1t_rep=w1t_rep.astype(bf16),
        w2t=w2t.astype(bf16),
        ident128=np.eye(128).astype(bf16),
        identT=np.eye(128, dtype=f32),
        repmat=repmat,
    )
    per_core = []
    for core in range(NCORES):
        n0 = core * NLOC
        xyz4_loc = np.concatenate([2.0 * xyz[:, n0:n0 + NLOC],
                                   -np.ones((1, NLOC), f32)], axis=0).astype(f32)
        per_core.append(dict(
            common,
            feats_loc=np.ascontiguousarray(feats_stack[:, :, n0:n0 + NLOC]),
            xyz4_loc=xyz4_loc,
            xyz_loc3=xyz[:, n0:n0 + NLOC].astype(bf16),
        ))
    return per_core


def assemble(outs):
    """outs: list of 8 arrays [128, 768] -> [1, 64, 1024, 12] fp32."""
    parts = []
    for o in outs:
        x = np.asarray(o, np.float32).reshape(2, 64, NAP, 128)
        parts.append(np.transpose(x, (1, 3, 2, 0)).reshape(64, 128, 12))
    return np.concatenate(parts, axis=1)[None].astype(np.float32)


def kernel(**inputs):
    nc = build_program()
    in_maps = host_prep(**inputs)
    res = run_bass_kernel_spmd(nc, in_maps, list(range(NCORES)))
    return assemble([res.results[i]["out"] for i in range(NCORES)])


# revision 22
# speedup vs baseline: 1.3130x; 1.0335x over previous
"""Trainium2 Bass kernel for nn_PointTransformerBatchLayer.

Strategy (8 NeuronCores, no cross-core communication):
  - Shard the point axis N=1024 -> 128 points per core. Every core receives
    the full xyz / feats (needed for KNN and neighbor gathers) plus its own
    local slices.
  - Per core: KNN via one fp32 matmul (s = 2 p_i.p_j - |p_j|^2) + top-16
    through 2 rounds of DVE max/max_index/match_replace.
  - qkv projected with bf16 matmuls; k/v for all anchors plus padded xyz are
    written to a DRAM row buffer [N, 12x(k64|v64) + xyz_pad] and gathered
    with dma_gather(transpose=True) in 4 x 512-idx calls (the SWDGE
    descriptor ring holds ~512 descriptors per instruction). One gather
    delivers every anchor's k/v and the neighbor xyz in the perfect
    (anchor-pair, channel)-on-partition layout.
  - Anchors processed in pairs ("apairs") so DVE/ACT run with 128 busy
    partitions. pe' = pos_mlp2@relu(pos_mlp1@anchor_rot(rel)) + q via
    block-diagonal weights + an identity-matmul broadcast of q.
  - softmax uses sum(attn)==1:  out = sum(e*(v_g+pe'))/sum(e) - q.
  - bf16 on the matmul path, fp16 on the exp/softmax path (0.5% rel err),
    PSUM fp32, KNN fp32.
"""
import sys

if "/opt/trn_rl_repo" not in sys.path:
    sys.path.insert(0, "/opt/trn_rl_repo")

import numpy as np

import concourse.bass as bass
import concourse.bacc as bacc
import concourse.tile as tile
import concourse.mybir as mybir
from concourse.bass_utils import run_bass_kernel_spmd

BF16 = mybir.dt.bfloat16
F16 = mybir.dt.float16
F32 = mybir.dt.float32
I16 = mybir.dt.int16
U16 = mybir.dt.uint16
AF = mybir.ActivationFunctionType
ALU = mybir.AluOpType

DIM, N, KNN, NA = 64, 1024, 16, 12
NCORES, NLOC, NAP = 8, 128, 6
NK = NLOC * KNN            # 2048 gathered elements per core
ROW = NAP * 256            # kv row: 12 anchors x (k64|v64) = 1536 elems
FC = 512                   # matmul free-dim chunk == gather chunk
NCHUNK = NK // FC          # 4 chunks
NQ = 4                     # SWDGE queues used for gathers

_PROG = None


def _declare(nc):
    p = {}
    def inp(name, shape, dt):
        p[name] = nc.declare_dram_parameter(name, list(shape), dt, isOutput=False)
    inp("feats_stack", (NAP, 128, N), BF16)
    inp("feats_loc", (NAP, 128, NLOC), BF16)
    inp("xyz4_all", (4, N), F32)
    inp("xyz4_loc", (4, NLOC), F32)
    inp("xyz_loc3", (3, NLOC), BF16)
    inp("xyz_rows", (N, 128), BF16)
    inp("pos1_w", (NAP, 3, 128), BF16)
    inp("w2blk", (128, 128), BF16)
    inp("wq_blk", (128, 128), BF16)
    inp("wkv_blk", (128, 256), BF16)
    inp("w1t_rep", (128, 256), BF16)
    inp("w2t", (128, 128), BF16)
    inp("ident128", (128, 128), BF16)
    inp("identT", (128, 128), F32)
    inp("repmat", (16, 128), F32)
    p["out"] = nc.declare_dram_parameter("out", [128, NAP * 128], F32, isOutput=True)
    return p


def _emit(tc, p):
    nc = tc.nc
    from contextlib import ExitStack

    with ExitStack() as ctx:
        const = ctx.enter_context(tc.tile_pool(name="const", bufs=1))
        big = ctx.enter_context(tc.tile_pool(name="big", bufs=1))
        work = ctx.enter_context(tc.tile_pool(name="work", bufs=3))
        work2 = ctx.enter_context(tc.tile_pool(name="work2", bufs=2))
        dram = ctx.enter_context(tc.tile_pool(name="dram", bufs=1, space="DRAM"))

        # ---- constant loads ----
        def load_const(name, shape, dt):
            t = const.tile(list(shape), dt, tag=name)
            nc.sync.dma_start(t[:], p[name][:])
            return t

        xyz4a_sb = load_const("xyz4_all", (4, N), F32)
        xyz4l_sb = load_const("xyz4_loc", (4, NLOC), F32)
        xyzl3_sb = load_const("xyz_loc3", (3, NLOC), BF16)
        idT_sb = load_const("identT", (128, 128), F32)
        rep_sb = load_const("repmat", (16, 128), F32)
        wkv_sb = load_const("wkv_blk", (128, 256), BF16)
        wqblk_sb = load_const("wq_blk", (128, 128), BF16)
        w2blk_sb = load_const("w2blk", (128, 128), BF16)
        w1t_sb = load_const("w1t_rep", (128, 256), BF16)
        w2t_sb = load_const("w2t", (128, 128), BF16)
        id16_sb = load_const("ident128", (128, 128), BF16)

        featsl_sb = const.tile([128, NAP, NLOC], BF16, tag="featsl")
        nc.sync.dma_start(featsl_sb[:], p["feats_loc"].rearrange("a p n -> p a n"))
        pos1_sb = const.tile([3, NAP, 128], BF16, tag="pos1")
        nc.sync.dma_start(pos1_sb[:], p["pos1_w"].rearrange("a p n -> p a n"))
        feats_sb = const.tile([128, NAP, N], BF16, tag="feats")
        for ap in range(NAP):
            nc.sync.dma_start(feats_sb[:, ap, :], p["feats_stack"][ap, :, :])

        kv_rows = dram.tile([N, ROW], BF16, tag="kv_rows")

        with tc.tile_pool(name="ps_pre", bufs=1, space="PSUM") as ps1, \
             tc.tile_pool(name="ps_qkv", bufs=2, space="PSUM") as psq:
            # ---- KNN ----
            s_ps = ps1.tile([128, N], F32, tag="s_ps")
            for i in range(2):
                nc.tensor.matmul(s_ps[:, i * 512:(i + 1) * 512], lhsT=xyz4l_sb[:],
                                 rhs=xyz4a_sb[:, i * 512:(i + 1) * 512],
                                 start=True, stop=True)
            s_sb = big.tile([128, N], F32, tag="s_sb")
            nc.scalar.activation(s_sb[:], s_ps[:], AF.Copy)

            m8a = big.tile([128, 8], F32, tag="m8a")
            m8b = big.tile([128, 8], F32, tag="m8b")
            idxu = big.tile([128, KNN], U16, tag="idxu")
            s2_sb = big.tile([128, N], F32, tag="s2_sb")
            nc.vector.max(m8a[:], s_sb[:])
            nc.vector.max_index(idxu[:, 0:8], m8a[:], s_sb[:])
            nc.vector.match_replace(s2_sb[:], m8a[:], s_sb[:], -1e30)
            nc.vector.max(m8b[:], s2_sb[:])
            nc.vector.max_index(idxu[:, 8:16], m8b[:], s2_sb[:])

            # idx -> wrapped [16,128] -> replicated [128,128] int16
            idxf = big.tile([128, KNN], F32, tag="idxf")
            nc.vector.tensor_copy(idxf[:], idxu[:])
            tp_ps = ps1.tile([16, 128], F32, tag="tp_ps")
            nc.tensor.transpose(tp_ps[:], idxf[:], idT_sb[:])
            idxT = big.tile([16, 128], F32, tag="idxT")
            nc.scalar.activation(idxT[:], tp_ps[:], AF.Copy)
            rep_ps = ps1.tile([128, 128], F32, tag="rep_ps")
            nc.tensor.matmul(rep_ps[:], lhsT=rep_sb[:], rhs=idxT[:], start=True, stop=True)
            idx_rep = big.tile([128, 128], I16, tag="idx_rep")
            nc.vector.tensor_copy(idx_rep[:], rep_ps[:])

            # ---- qkv ----
            q_sb = big.tile([128, NAP, NLOC], BF16, tag="q_sb")
            for ap in range(NAP):
                q_ps = psq.tile([128, NLOC], F32, tag="q_ps")
                nc.tensor.matmul(q_ps[:], lhsT=wqblk_sb[:], rhs=featsl_sb[:, ap, :],
                                 start=True, stop=True)
                nc.scalar.activation(q_sb[:, ap, :], q_ps[:], AF.Copy)

                stage = work2.tile([128, 8, 256], BF16, tag="stage")
                for cc in range(4):  # 2 n-chunks of 128 per psum tile
                    kv_ps = psq.tile([128, 512], F32, tag="kv_ps")
                    for j in range(2):
                        c = cc * 2 + j
                        nc.tensor.matmul(kv_ps[:, j * 256:(j + 1) * 256],
                                         lhsT=feats_sb[:, ap, c * 128:(c + 1) * 128],
                                         rhs=wkv_sb[:], start=True, stop=True)
                    nc.scalar.activation(stage[:, cc * 2:(cc + 1) * 2, :], kv_ps[:], AF.Copy)
                nc.sync.dma_start(
                    kv_rows.rearrange("(c p) (a e) -> p c a e", p=128, e=256)[:, :, ap, :],
                    stage[:])

        # ---- gathers ----
        # SWDGE ring fits ~512 TX + ~1024 RX descriptors per instruction, so
        # every gather is a 512-idx chunk; calls round-robin the 4 SWDGE
        # queues, whose Q7 core-pairs generate descriptors in parallel.
        qrr = [0]
        def next_q():
            q = qrr[0] % NQ
            qrr[0] += 1
            return q

        xyzg = big.tile([128, NK], BF16, tag="xyzg")
        for cc in range(NCHUNK):
            nc.gpsimd.dma_gather(
                out_ap=xyzg[:, cc * FC:(cc + 1) * FC].rearrange("p (o j) -> p o j", o=1),
                in_ap=p["xyz_rows"][:, :],
                idxs_ap=idx_rep[:, cc * (FC // 16):(cc + 1) * (FC // 16)],
                num_idxs=FC, num_idxs_reg=FC, elem_size=128,
                transpose=True, queue_num=next_q())

        # per-(apair, chunk) gather tiles [128, 2*FC]: [:, :FC]=k, [:, FC:]=v
        kvg = [[None] * NCHUNK for _ in range(NAP)]
        for ap in range(NAP):
            for cc in range(NCHUNK):
                t = work2.tile([128, 2 * FC], BF16, tag=f"kvg{ap % 3}_{cc}")
                nc.gpsimd.dma_gather(
                    out_ap=t.rearrange("p (o j) -> p o j", o=2),
                    in_ap=kv_rows[:, ap * 256:(ap + 1) * 256],
                    idxs_ap=idx_rep[:, cc * (FC // 16):(cc + 1) * (FC // 16)],
                    num_idxs=FC, num_idxs_reg=FC, elem_size=256, elem_step=ROW,
                    transpose=True, queue_num=next_q())
                kvg[ap][cc] = t

        # ---- rel = xyz_loc (bcast over k) - gathered xyz ----
        nw = FC // KNN  # 32 points per chunk
        rel_sb = big.tile([3, NK], BF16, tag="rel")
        nc.vector.tensor_sub(
            rel_sb.rearrange("p (n k) -> p n k", k=KNN),
            xyzl3_sb.rearrange("p n -> p n ()").broadcast_to([3, NLOC, KNN]),
            xyzg[0:3, :].rearrange("p (n k) -> p n k", k=KNN))

        out_sb = big.tile([128, NAP * 128], F32, tag="out_sb")
        S1_all = big.tile([128, NAP * 128], F32, tag="S1_all")
        S0_all = big.tile([128, NAP * 128], F32, tag="S0_all")

        def tree16(dst, src):
            """dst[p, n] = sum over k of src[p, (n k)]; fp16 in, fp32 out."""
            a8 = work.tile([128, NLOC * 8], F16, tag="tr8")
            nc.vector.tensor_add(
                a8.rearrange("p (n k) -> p n k", k=8),
                src.rearrange("p (n k) -> p n k", k=KNN)[:, :, 0:8],
                src.rearrange("p (n k) -> p n k", k=KNN)[:, :, 8:16])
            a4 = work.tile([128, NLOC * 4], F16, tag="tr4")
            nc.vector.tensor_add(
                a4.rearrange("p (n k) -> p n k", k=4),
                a8.rearrange("p (n k) -> p n k", k=8)[:, :, 0:4],
                a8.rearrange("p (n k) -> p n k", k=8)[:, :, 4:8])
            a2 = work.tile([128, NLOC * 2], F16, tag="tr2")
            nc.vector.tensor_add(
                a2.rearrange("p (n k) -> p n k", k=2),
                a4.rearrange("p (n k) -> p n k", k=4)[:, :, 0:2],
                a4.rearrange("p (n k) -> p n k", k=4)[:, :, 2:4])
            nc.vector.tensor_add(
                dst,
                a2.rearrange("p (n k) -> p n k", k=2)[:, :, 0:1].rearrange("p n k -> p (n k)"),
                a2.rearrange("p (n k) -> p n k", k=2)[:, :, 1:2].rearrange("p n k -> p (n k)"))

        # ---- main loop: 24 (apair, chunk) steps, software-pipelined so the
        # pe-path of step i and the attn-path of step i-1 are emitted
        # adjacently -> the PE queue interleaves independent matmul groups
        # and never idles on vector-engine evacuations.
        with tc.tile_pool(name="ps_m1", bufs=2, space="PSUM") as psm1, \
             tc.tile_pool(name="ps_m2", bufs=1, space="PSUM") as psmh:
            chunks = [(ap, c) for ap in range(NAP) for c in range(NCHUNK)]
            e_tiles, vgp_tiles, sim_q, pep_q = {}, {}, {}, {}

            def pe_path(ap, c):
                cs = slice(c * FC, (c + 1) * FC)
                if c == 0:
                    e_tiles[ap] = work2.tile([128, NK], F16, tag="e_sb", name=f"e_sb{ap}")
                    vgp_tiles[ap] = work2.tile([128, NK], F16, tag="vgp_sb", name=f"vgp_sb{ap}")
                kg = kvg[ap][c][:, 0:FC]
                vg = kvg[ap][c][:, FC:2 * FC]
                pe1_ps = psm1.tile([128, FC], F32, tag="pp")
                nc.tensor.matmul(pe1_ps[:], lhsT=pos1_sb[:, ap, :], rhs=rel_sb[:, cs],
                                 start=True, stop=True)
                relu1 = work.tile([128, FC], BF16, tag="relu1")
                nc.scalar.activation(relu1[:], pe1_ps[:], AF.Relu)
                # pe' = W2blk @ relu1 + I @ q_bcast   (stays in PSUM)
                pep_ps = psm1.tile([128, FC], F32, tag="pp")
                nc.tensor.matmul(pep_ps[:], lhsT=w2blk_sb[:], rhs=relu1[:],
                                 start=True, stop=False)
                qb = q_sb[:, ap, c * nw:(c + 1) * nw] \
                    .rearrange("p n -> p n ()").broadcast_to([128, nw, KNN])
                nc.tensor.matmul(pep_ps.rearrange("p (n k) -> p n k", k=KNN),
                                 lhsT=id16_sb[:], rhs=qb, start=False, stop=True)
                # sim = pe' - k_g ; vgp = v_g + pe'  (both read pe' PSUM)
                sim = work.tile([128, FC], BF16, tag="sim")
                nc.vector.scalar_tensor_tensor(
                    sim[:], pep_ps[:], 0.0, kg, ALU.add, ALU.subtract)
                nc.vector.scalar_tensor_tensor(
                    vgp_tiles[ap][:, cs], pep_ps[:], 0.0, vg, ALU.add, ALU.add)
                sim_q[(ap, c)] = sim

            def attn_path(ap, c):
                cs = slice(c * FC, (c + 1) * FC)
                sim = sim_q.pop((ap, c))
                s2_ps = psm1.tile([128, FC], F32, tag="s2_ps")
                for s in range(2):
                    h_ps = psmh.tile([128, 2 * FC], F32, tag=f"hp{s}")
                    for h in range(2):
                        nc.tensor.matmul(
                            h_ps[:, h * FC:(h + 1) * FC],
                            lhsT=w1t_sb[64 * h:64 * (h + 1), 128 * s:128 * (s + 1)],
                            rhs=sim[64 * h:64 * (h + 1), :],
                            start=True, stop=True)
                    hr = work.tile([128, 2 * FC], BF16, tag=f"hr{s}")
                    if (c + s) % 2 == 0:
                        nc.scalar.activation(hr[:], h_ps[:], AF.Relu)
                    else:
                        nc.vector.tensor_scalar_max(hr[:], h_ps[:], 0.0)
                    for h in range(2):
                        nc.tensor.matmul(s2_ps[64 * h:64 * (h + 1), :],
                                         lhsT=w2t_sb[:, 64 * s:64 * (s + 1)],
                                         rhs=hr[:, h * FC:(h + 1) * FC],
                                         start=(s == 0), stop=(s == 1),
                                         skip_group_check=True)
                nc.scalar.activation(e_tiles[ap][:, cs], s2_ps[:], AF.Exp)

            def aggregate(ap):
                # S1 = sum_k e*vgp, S0 = sum_k e; out = S1/S0 - q
                aps = slice(ap * 128, (ap + 1) * 128)
                e_sb, vgp_sb = e_tiles.pop(ap), vgp_tiles.pop(ap)
                t_sb = work2.tile([128, NK], F16, tag="t_sb")
                nc.vector.tensor_mul(t_sb[:], e_sb[:], vgp_sb[:])
                tree16(S1_all[:, aps], t_sb)
                tree16(S0_all[:, aps], e_sb)
                nc.vector.reciprocal(S0_all[:, aps], S0_all[:, aps])
                nc.vector.tensor_mul(S1_all[:, aps], S1_all[:, aps], S0_all[:, aps])
                nc.vector.tensor_sub(out_sb[:, aps], S1_all[:, aps], q_sb[:, ap, :])
                nc.sync.dma_start(p["out"][:, ap * 128:(ap + 1) * 128], out_sb[:, aps])

            for i in range(len(chunks) + 1):
                if i < len(chunks):
                    pe_path(*chunks[i])
                if i >= 1:
                    ap, c = chunks[i - 1]
                    attn_path(ap, c)
                    if c == NCHUNK - 1:
                        aggregate(ap)


def build_program():
    global _PROG
    if _PROG is not None:
        return _PROG
    nc = bacc.Bacc("TRN2", target_bir_lowering=False, debug=False,
                   num_swdge_queues=NQ)
    p = _declare(nc)
    with tile.TileContext(nc) as tc:
        _emit(tc, p)
    nc.compile()
    _PROG = nc
    return nc


def host_prep(xyz, feats, anchors, to_qkv, pos_mlp1, pos_mlp2, attn_mlp1, attn_mlp2):
    import ml_dtypes
    bf16, f32 = ml_dtypes.bfloat16, np.float32
    xyz = np.asarray(xyz, f32)[0]        # [3, N]
    feats = np.asarray(feats, f32)[0]    # [DIM, N, NA]
    anchors = np.asarray(anchors, f32)
    to_qkv = np.asarray(to_qkv, f32)
    pos_mlp1 = np.asarray(pos_mlp1, f32)
    pos_mlp2 = np.asarray(pos_mlp2, f32)
    attn_mlp1 = np.asarray(attn_mlp1, f32)
    attn_mlp2 = np.asarray(attn_mlp2, f32)

    Wq, Wk, Wv = to_qkv[:DIM], to_qkv[DIM:2 * DIM], to_qkv[2 * DIM:]
    Wa = np.einsum("hj,aji->ahi", pos_mlp1, anchors)     # [NA, 64, 3]

    pos1_w = np.stack([np.concatenate([Wa[2 * ap].T, Wa[2 * ap + 1].T], axis=1)
                       for ap in range(NAP)]).astype(bf16)
    w2blk = np.zeros((128, 128), f32)
    w2blk[:64, :64] = pos_mlp2.T
    w2blk[64:, 64:] = pos_mlp2.T
    wq_blk = np.zeros((128, 128), f32)
    wq_blk[:64, :64] = Wq.T
    wq_blk[64:, 64:] = Wq.T
    wkv_blk = np.zeros((128, 256), f32)
    wkv_blk[:64, 0:64] = Wk.T
    wkv_blk[64:, 64:128] = Wk.T
    wkv_blk[:64, 128:192] = Wv.T
    wkv_blk[64:, 192:256] = Wv.T
    w1t_rep = np.concatenate([attn_mlp1.T, attn_mlp1.T], axis=0)
    w2t = np.zeros((128, 128), f32)
    w2t[:, :64] = attn_mlp2.T[:128]
    w2t[:, 64:] = attn_mlp2.T[128:]
    repmat = np.zeros((16, 128), f32)
    repmat[np.arange(128) % 16, np.arange(128)] = 1.0

    feats_stack = np.zeros((NAP, 128, N), f32)
    for ap in range(NAP):
        feats_stack[ap, :64] = feats[:, :, 2 * ap]
        feats_stack[ap, 64:] = feats[:, :, 2 * ap + 1]
    feats_stack = feats_stack.astype(bf16)

    xyz_rows = np.zeros((N, 128), f32)
    xyz_rows[:, :3] = xyz.T
    xyz_rows = xyz_rows.astype(bf16)

    sq = np.sum(xyz * xyz, axis=0)
    xyz4_all = np.concatenate([xyz, sq[None]], axis=0).astype(f32)

    common = dict(
        feats_stack=feats_stack,
        xyz4_all=xyz4_all,
        xyz_rows=xyz_rows,
        pos1_w=pos1_w,
        w2blk=w2blk.astype(bf16),
        wq_blk=wq_blk.astype(bf16),
        wkv_blk=wkv_blk.astype(bf16),
        w1t_rep=w1t_rep.astype(bf16),
        w2t=w2t.astype(bf16),
        ident128=np.eye(128).astype(bf16),
        identT=np.eye(128, dtype=f32),
        repmat=repmat,
    )
    per_core = []
    for core in range(NCORES):
        n0 = core * NLOC
        xyz4_loc = np.concatenate([2.0 * xyz[:, n0:n0 + NLOC],
                                   -np.ones((1, NLOC), f32)], axis=0).astype(f32)
        per_core.append(dict(
            common,
            feats_loc=np.ascontiguousarray(feats_stack[:, :, n0:n0 + NLOC]),
            xyz4_loc=xyz4_loc,
            xyz_loc3=xyz[:, n0:n0 + NLOC].astype(bf16),
        ))
    return per_core


def assemble(outs):
    """outs: list of 8 arrays [128, 768] -> [1, 64, 1024, 12] fp32."""
    parts = []
    for o in outs:
        x = np.asarray(o, np.float32).reshape(2, 64, NAP, 128)
        parts.append(np.transpose(x, (1, 3, 2, 0)).reshape(64, 128, 12))
    return np.concatenate(parts, axis=1)[None].astype(np.float32)


def kernel(**inputs):
    nc = build_program()
    in_maps = host_prep(**inputs)
    res = run_bass_kernel_spmd(nc, in_maps, list(range(NCORES)))
    return assemble([res.results[i]["out"] for i in range(NCORES)])
